# revision 2
# baseline (speedup 1.0000x reference)
"""Multi-head attention (B=4,N=2048,C=768,H=12) on 8 trn2 NeuronCores.

Sharding: core = (batch b, head-half p): 6 heads of one batch per core;
host sums the two half-partials per batch and adds the bias.

Structure (vs the 281us v1 baseline; now ~236us):
 - AV matmul flipped: out[q,d] = est[kv,q].T @ v[kv,d+1] with est as the
   stationary operand -> out free = 65 instead of 512, so attention@V
   costs 42us of PE instead of 83us. The appended ones column makes the
   softmax denominator fall out of column 64.
 - Normalize is a per-partition tensor_scalar on the [q,d] tile (the
   denominator is a per-q scalar); a PE transpose ([q,d]->[d,q], 128
   cycles, odd head straight into PSUM partitions 64-127 via
   tile_position) rebuilds the proj layout.
 - proj accumulates all 3 head-pairs on-device; y ships as bf16.
 - x/wqkv stream as bf16 (cost model charges the moving operand; f32r
   needs >=256-wide outputs to avoid a 4x penalty) and the input fill is
   7 wide DMAs (the sim serializes a ~625ns HWDGE stage per DMA and all
   transfers share one DMA_ENGINES device).
 - Softmax exp mostly on ACT; a per-strip subset of kv tiles uses a
   Schraudolph bit-trick exp on DVE (bf16(int16(s*A+B)), rms 1.8%,
   zero-mean) to keep ACT off the critical path.
 - Flat software-pipelined slot loop over (pair, strip, kv-tile): slot i
   emits AV for seq[i-3], one deferred normalize/transpose/proj item,
   scores+exp for seq[i+3], then deadline/budget-paced qkproj/v-proj
   fillers, so PE (the pacing engine; idle gaps also downclock it) never
   queues an unresolved wait.

PSUM (8 banks): sc 2x[128,2,512]f32 (4) + av [128,2,4,128]f32 (2,
single buf, 512B slot stride so no accumulation group crosses a 2KB
bank; start=True only on each bank's first write since it marks the
whole bank pending-zero) + tp [128,128]bf16 (1) + scratch [128,512]f32
(1, shared by warmup/qkproj/v/proj via one tag).
"""

import sys
from collections import deque

import numpy as np
import ml_dtypes

_REPO = "/opt/trn_rl_repo"
if _REPO not in sys.path:
    sys.path.insert(0, _REPO)

import concourse.bacc as bacc
import concourse.mybir as mybir
import concourse.tile as tile
from concourse.bass_utils import run_bass_kernel_spmd
from concourse.masks import make_identity

B, N, C, H, D = 4, 2048, 768, 12, 64
HL = H // 2          # heads per core
NP = HL // 2         # head pairs per core (3)
SCALE = D ** -0.5
NCORES = 8
KT_C = C // 128      # 6 contraction tiles over C
KVT = N // 128       # 16 kv tiles
NS = N // 512        # 4 query strips of 512

F32 = mybir.dt.float32
F32R = mybir.dt.float32r
BF16 = mybir.dt.bfloat16
I16 = mybir.dt.int16
EXP = mybir.ActivationFunctionType.Exp
MULT = mybir.AluOpType.mult
ADD = mybir.AluOpType.add

# Schraudolph bit-trick exp on DVE: bf16(int16(s*A + B)) ~ exp(s*SCALE).
# rms rel err 1.8%, near-zero mean (c=-7.5); used on a few kv tiles per
# strip to keep ACT off the critical path (PE must stay saturated: any
# PE idle gap drops its clock to 1.2GHz for the next 3us).
SCHRAU_A = 128 * np.log2(np.e) * SCALE
SCHRAU_B = 127.0 * 128 - 7.5
# kv tiles whose exp runs on DVE instead of ACT, by (pair, strip):
# pair0 is PE-overloaded (v-proj + deadline qk chunks), so no offload
# there; pair1/pair2 strips offload 2-3 tiles to keep ACT off the
# critical path while PE stays saturated
SCHRAU_PLAN = {0: ((8,),) * 4,
               1: ((0, 2, 5, 8, 11, 14),) * 4,
               2: ((0, 2, 5, 8, 11, 14),) * 4}


def _schrau_kts(pr, s):
    return SCHRAU_PLAN[pr][s]

_CACHE = {}


def _build():
    nc = bacc.Bacc("TRN2", target_bir_lowering=False, debug=False,
                   num_devices=NCORES)
    xT = nc.dram_tensor("xT", [C, N], BF16, kind="ExternalInput").ap()
    wqkT = nc.dram_tensor("wqkT", [C, 2 * HL * D], BF16, kind="ExternalInput").ap()
    wvT = nc.dram_tensor("wvT", [C, HL * D], BF16, kind="ExternalInput").ap()
    wpT = nc.dram_tensor("wpT", [HL * D, C], BF16, kind="ExternalInput").ap()
    y = nc.dram_tensor("y", [N, C], BF16, kind="ExternalOutput").ap()

    with tile.TileContext(nc) as tc:
        with (
            tc.tile_pool(name="singles", bufs=1) as singles,
            tc.tile_pool(name="sc_p", bufs=2, space="PSUM") as sc_p,
            tc.tile_pool(name="av_p", bufs=1, space="PSUM") as av_p,
            tc.tile_pool(name="tp_p", bufs=1, space="PSUM") as tp_p,
            tc.tile_pool(name="scr_p", bufs=1, space="PSUM") as scr_p,
            tc.tile_pool(name="est_p", bufs=4) as est_p,
            tc.tile_pool(name="stg_p", bufs=6) as stg_p,
            tc.tile_pool(name="rec_p", bufs=2) as rec_p,
            tc.tile_pool(name="ysb_p", bufs=3) as ysb_p,
        ):
            xT_sb = singles.tile([128, KT_C, N], BF16)
            wqk_sb = singles.tile([128, KT_C, 2 * HL * D], BF16)
            wv_sb = singles.tile([128, KT_C, HL * D], BF16)
            wp_sb = singles.tile([128, NP, C], BF16)
            qk_sb = singles.tile([128, 2 * NP, N], F32R)
            v_sb = singles.tile([128, KVT, HL, D + 1], BF16)
            attn_sb = singles.tile([128, NP, N], BF16)
            ident = singles.tile([128, 128], BF16)
            warm_sb = singles.tile([128, 640], BF16)

            # The sim serializes a ~625ns HWDGE stage per DMA instruction,
            # so batch the fill into 7 wide DMAs. xT goes chunk-major on
            # SP's queue (first score work unblocks after ~3.5us); weights
            # go on ACT's queue.
            xT_r = xT.rearrange("(k p) n -> p k n", k=KT_C)
            for c in range(NS):
                csl = slice(c * 512, (c + 1) * 512)
                nc.sync.dma_start(xT_sb[:, :, csl], xT_r[:, :, csl])
            wqk_r = wqkT.rearrange("(k p) n -> p k n", k=KT_C)
            nc.scalar.dma_start(wqk_sb[:, :, HL * D:], wqk_r[:, :, HL * D:])
            nc.scalar.dma_start(wqk_sb[:, :, 0:HL * D], wqk_r[:, :, 0:HL * D])
            nc.scalar.dma_start(wv_sb, wvT.rearrange("(k p) n -> p k n", k=KT_C))
            nc.scalar.dma_start(wp_sb, wpT.rearrange("(k p) n -> p k n", k=NP))
            nc.vector.memset(warm_sb, 0.0)
            nc.vector.memset(v_sb[:, :, :, D:D + 1], 1.0)
            make_identity(nc, ident)

            # warm the ACT exp table (hardware-only cost; sim ignores it)
            warm_in = rec_p.tile([1, 2], F32, tag="warm")
            warm_out = rec_p.tile([1, 2], BF16, tag="warmo")
            nc.vector.memset(warm_in, 0.0)
            nc.scalar.activation(warm_out, warm_in, EXP, scale=SCALE)

            # PE p-state ramp warmers while the xT DMA fill lands
            for _ in range(10):
                warm_ps = scr_p.tile([128, 512], F32, tag="scr")
                nc.tensor.matmul(warm_ps, lhsT=warm_sb[:, 0:128],
                                 rhs=warm_sb[:, 128:640])

            # ---- filler emission (qkproj / v-proj paced into attention) ----
            def emit_qk_chunk(t, c, via_act=False):
                """qk tile t (0-2: q pairs, 3-5: k pairs), 512-col chunk c.
                via_act stages the PSUM->SBUF copy on the scalar engine,
                which has slack in the exp-offloaded pairs, keeping DVE's
                queue short (the offloaded exp sits in the score-ring WAR
                chain, so DVE latency there stalls PE)."""
                ps = scr_p.tile([128, 512], F32, tag="scr")
                csl = slice(c * 512, (c + 1) * 512)
                for kt in range(KT_C):
                    nc.tensor.matmul(
                        ps,
                        lhsT=wqk_sb[:, kt, t * 128:(t + 1) * 128],
                        rhs=xT_sb[:, kt, csl],
                        start=(kt == 0), stop=(kt == KT_C - 1),
                    )
                if via_act:
                    nc.scalar.copy(qk_sb[:, t, csl], ps)
                else:
                    nc.vector.tensor_copy(qk_sb[:, t, csl], ps)

            def emit_v_mt(pr, mt):
                """v rows for kv tile mt, all 6 heads (pr unused; f32r
                needs a >=256-wide moving operand to stream 1 cycle/row)."""
                ps = scr_p.tile([128, 512], F32, tag="scr")
                for kt in range(KT_C):
                    nc.tensor.matmul(
                        ps[:, 0:384],
                        lhsT=xT_sb[:, kt, mt * 128:(mt + 1) * 128],
                        rhs=wv_sb[:, kt, :],
                        start=(kt == 0), stop=(kt == KT_C - 1),
                    )
                nc.vector.tensor_copy(
                    v_sb[:, mt, :, 0:D],
                    ps[:, 0:384].rearrange("p (h d) -> p h d", h=HL),
                )

            # fillers carry a deadline slot: q-tile chunk c of pair pr is
            # read starting at slot 16*(4*pr+c)-2; k-tile chunk c at slot
            # 64*pr+4*c-2. Budget pacing pulls them earlier when PE has
            # slack; the deadline forces emission when it hasn't.
            QK_CYC, V_CYC = KT_C * 512 + 533, KT_C * 384 + 533
            items = []
            for c in (1, 2, 3):
                # k chunks of pair0, consumed by sc(0,0,4c) at slot 4c-2;
                # paced to match the chunk-major xT DMA arrivals
                items.append((4 * c - 3, QK_CYC, emit_qk_chunk, (3, c)))
                items.append((16 * c - 4, QK_CYC, emit_qk_chunk, (0, c)))
            for c in range(NS):
                items.append((58 + 16 * c, QK_CYC, emit_qk_chunk, (1, c)))
                items.append((52 + 4 * c, QK_CYC, emit_qk_chunk, (4, c)))
                items.append((100 + 4 * c, QK_CYC, emit_qk_chunk, (2, c)))
                items.append((96 + 2 * c, QK_CYC, emit_qk_chunk, (5, c)))
            items.sort(key=lambda it: it[0])
            fillers = deque(items)

            # deferred work carried across strips: transposes of the
            # previous strip (list of fns), proj of the previous strip
            pending = deque()

            def emit_transpose(stg01, pr, s, q2):
                """Transpose both heads' [q,64] tiles into one [128,128]
                psum tile (h1 straight to partitions 64-127), then copy."""
                tp = tp_p.tile([128, 128], BF16, tag="tp")
                for h in range(2):
                    out = tp[64 * h:64 * h + 64, :]
                    nc.tensor.matmul(out, lhsT=stg01[h], rhs=ident,
                                     is_transpose=True,
                                     tile_position=(0, 64 * h))
                csl = slice(s * 512 + q2 * 128, s * 512 + (q2 + 1) * 128)
                nc.vector.tensor_copy(attn_sb[:, pr, csl], tp)

            proj_ysb = {}
            ysb_tail = singles.tile([128, 4, 2, 384], BF16, name="ysb_tail")

            def emit_proj_half(mt, ns_, tail=False):
                if ns_ == 0 and not tail:
                    ysb_new = ysb_p.tile([128, 2, 384], BF16, tag="ysb",
                                         name="ysb")
                    proj_ysb[mt] = ysb_new
                ysb = proj_ysb.get(mt)
                if tail:
                    # scores are done by now: borrow the (idle) sc banks so
                    # the final strip's proj halves double-buffer instead of
                    # serializing on the single scr bank; stage via ACT
                    # (also idle) and ship one batched y DMA at the end
                    sct = sc_p.tile([128, 2, 512], F32, tag="sc", name="sc")
                    yp = sct[:, 0, :]
                else:
                    yp = scr_p.tile([128, 512], F32, tag="scr")
                for pr in range(NP):
                    nc.tensor.matmul(
                        yp[:, 0:384],
                        lhsT=attn_sb[:, pr, mt * 128:(mt + 1) * 128],
                        rhs=wp_sb[:, pr, ns_ * 384:(ns_ + 1) * 384],
                        start=(pr == 0), stop=(pr == NP - 1),
                    )
                if tail:
                    mi = mt - (NS - 1) * 4
                    nc.scalar.copy(ysb_tail[:, mi, ns_, :], yp[:, 0:384])
                    if mi == 1 and ns_ == 1:
                        nc.sync.dma_start(
                            y.rearrange("(m p) c -> p m c", m=KVT)[:, 12:14, :],
                            ysb_tail[:, 0:2].rearrange("p m a b -> p m (a b)"))
                    elif mi == 3 and ns_ == 1:
                        nc.sync.dma_start(
                            y.rearrange("(m p) c -> p m c", m=KVT)[:, 14:16, :],
                            ysb_tail[:, 2:4].rearrange("p m a b -> p m (a b)"))
                    return
                nc.vector.tensor_copy(ysb[:, ns_, :], yp[:, 0:384])
                if ns_ == 1:
                    nc.sync.dma_start(y[mt * 128:(mt + 1) * 128, :],
                                      ysb.rearrange("p a b -> p (a b)"))
                    del proj_ysb[mt]

            # ---- prefill: only chunk-0-dependent work, so the first
            # scores can issue as soon as the first 6 xT DMAs land ----
            emit_qk_chunk(3, 0)              # k tile of pair0, kv 0-3
            emit_qk_chunk(0, 0)              # q tile of pair0, strip0
            v0_emitted = 0

            # ---- attention: flat software-pipelined slot loop ----
            # slot i emits: av for seq[i-2] (its est finished a slot ago),
            # one deferred transpose/proj item, paced fillers, then the
            # scores+exp for seq[i+2]. PE thus never queues an unresolved
            # wait: everything it issues became ready >= 1 slot earlier.
            seq = [(pr, s, kt) for pr in range(NP) for s in range(NS)
                   for kt in range(KVT)]
            STRIP_CAP = globals().get('CAP', 4500)
            ests = {}
            avs = {}

            def emit_scores(pr, s, kt):
                sc = sc_p.tile([128, 2, 512], F32, tag="sc", name="sc")
                qsl = slice(s * 512, (s + 1) * 512)
                for h in range(2):
                    p0, p1 = 64 * h, 64 * h + 64
                    nc.tensor.matmul(
                        sc[:, h, :],
                        lhsT=qk_sb[p0:p1, NP + pr, kt * 128:(kt + 1) * 128],
                        rhs=qk_sb[p0:p1, pr, qsl],
                    )
                if kt in _schrau_kts(pr, s):
                    ei = est_p.tile([128, 2, 512], I16, tag="esti",
                                    bufs=5, name="ei")
                    nc.vector.tensor_scalar(
                        ei, sc, float(SCHRAU_A), float(SCHRAU_B),
                        MULT, ADD)
                    ests[(pr, s, kt)] = ei.bitcast(BF16)
                else:
                    est = est_p.tile([128, 2, 512], BF16, tag="est",
                                     bufs=globals().get("EST_BUFS", 8), name="est")
                    nc.scalar.activation(est, sc, EXP, scale=SCALE)
                    ests[(pr, s, kt)] = est

            def emit_av(pr, s, kt):
                est = ests.pop((pr, s, kt))
                if kt == 0:
                    avs[(pr, s)] = av_p.tile([128, 2, 4, 128], F32,
                                             tag="av", name="av")
                av = avs[(pr, s)]
                for h in range(2):
                    for q2 in range(4):
                        # start only on the bank's first write: a start=True
                        # matmul marks its whole 2KB PSUM bank pending-zero,
                        # so sibling slots must accumulate with start=False
                        nc.tensor.matmul(
                            av[:, h, q2, 0:D + 1],
                            lhsT=est[:, h, q2 * 128:(q2 + 1) * 128],
                            rhs=v_sb[:, kt, 2 * pr + h, :],
                            start=(kt == 0 and q2 == 0),
                            stop=(kt == KVT - 1),
                            skip_group_check=True,
                        )

            def finish_strip(pr, s, last=False):
                """Free the av banks with one copy, then normalize from
                SBUF and queue transposes (+ proj for the last pair).
                For the final strip everything is emitted inline, pipelined
                per q-subtile so the drain chain overlaps."""
                av = avs.pop((pr, s))
                avsb = stg_p.tile([128, 2, 4, D + 1], F32, tag="avsb",
                                  bufs=2, name="avsb")
                nc.vector.tensor_copy(avsb, av[:, :, :, 0:D + 1])
                rec = rec_p.tile([128, 2, 4, 1], F32, tag="rec")
                nc.vector.reciprocal(rec, avsb[:, :, :, D:D + 1])
                stga = stg_p.tile([128, 2, 4, D], BF16, tag="stg",
                                  bufs=2, name="stga")

                def norm_tp(q2):
                    for h in range(2):
                        nc.vector.tensor_scalar(
                            stga[:, h, q2, :], avsb[:, h, q2, 0:D],
                            rec[:, h, q2, :], None, MULT)
                    emit_transpose([stga[:, 0, q2, :], stga[:, 1, q2, :]],
                                   pr, s, q2)

                if last:
                    for q2 in range(4):
                        norm_tp(q2)
                    for mt in (s * 4, s * 4 + 1):
                        for ns_ in range(2):
                            emit_proj_half(mt, ns_, tail=True)
                    for mt in (s * 4 + 2, s * 4 + 3):
                        for ns_ in range(2):
                            emit_proj_half(mt, ns_, tail=True)
                    return
                for q2 in range(4):
                    pending.append((norm_tp, (q2,)))
                if pr == NP - 1:
                    for mt in range(s * 4, s * 4 + 4):
                        for ns_ in range(2):
                            pending.append((emit_proj_half, (mt, ns_)))

            LAG = globals().get('AV_LAG', 3)
            for j in range(LAG):
                emit_scores(*seq[j])
            spent = 0
            for i, trip in enumerate(seq):
                pr, s, kt = trip
                if i >= LAG:
                    emit_av(*seq[i - LAG])
                    opr, os_, okt = seq[i - LAG]
                    if okt == KVT - 1:
                        finish_strip(opr, os_)
                if pending:
                    fn, args = pending.popleft()
                    fn(*args)
                if i + LAG < len(seq):
                    emit_scores(*seq[i + LAG])
                # just-in-time v rows, behind the score stream
                if v0_emitted < KVT and i < KVT:
                    emit_v_mt(0, v0_emitted)
                    v0_emitted += 1
                budget = (i + 1) * STRIP_CAP // KVT
                while fillers and (fillers[0][0] <= i or spent < budget):
                    dl, cyc, fn, args = fillers.popleft()
                    fn(*args)
                    spent += cyc
            for j in range(LAG, 0, -1):
                pj = seq[len(seq) - j]
                if pj[2] == KVT - 1:
                    emit_av(*pj)
                else:
                    emit_av(*pj)
            finish_strip(NP - 1, NS - 1, last=True)
            while pending:
                fn, args = pending.popleft()
                if fn is emit_proj_half:
                    fn(*args, tail=True)
                else:
                    fn(*args)
            while fillers:
                dl, cyc, fn, args = fillers.popleft()
                fn(*args)

    nc.compile()
    return nc


def _get_nc():
    if "nc" not in _CACHE:
        _CACHE["nc"] = _build()
    return _CACHE["nc"]


def _prep_inputs(x, w_qkv, w_proj):
    """Per-core input dicts. Core c: batch c//2, head-half c%2."""
    wq, wk, wv = w_qkv[0:C], w_qkv[C:2 * C], w_qkv[2 * C:3 * C]
    in_maps = []
    for core in range(NCORES):
        b, p = divmod(core, 2)
        heads = [p * HL + j for j in range(HL)]
        qk_rows = np.concatenate(
            [wq[h * D:(h + 1) * D] for h in heads]
            + [wk[h * D:(h + 1) * D] for h in heads], axis=0)   # [768, C]
        v_rows = np.concatenate(
            [wv[h * D:(h + 1) * D] for h in heads], axis=0)     # [384, C]
        p_cols = np.concatenate(
            [w_proj[:, h * D:(h + 1) * D] for h in heads], axis=1)  # [C, 384]
        in_maps.append({
            "xT": np.ascontiguousarray(x[b].T).astype(ml_dtypes.bfloat16),
            "wqkT": np.ascontiguousarray(qk_rows.T).astype(ml_dtypes.bfloat16),
            "wvT": np.ascontiguousarray(v_rows.T).astype(ml_dtypes.bfloat16),
            "wpT": np.ascontiguousarray(p_cols.T).astype(ml_dtypes.bfloat16),
        })
    return in_maps


def kernel(x, w_qkv, w_proj, b_proj, _trace=False):
    x = np.asarray(x, dtype=np.float32)
    w_qkv = np.asarray(w_qkv, dtype=np.float32)
    w_proj = np.asarray(w_proj, dtype=np.float32)
    b_proj = np.asarray(b_proj, dtype=np.float32)

    nc = _get_nc()
    in_maps = _prep_inputs(x, w_qkv, w_proj)
    last_exc = None
    for _attempt in range(3):
        try:
            res = run_bass_kernel_spmd(nc, in_maps,
                                       core_ids=list(range(NCORES)),
                                       trace=_trace)
            break
        except Exception as e:
            last_exc = e
    else:
        raise last_exc
    _CACHE["last_results"] = res

    out = np.empty((B, N, C), dtype=np.float32)
    for b in range(B):
        out[b] = (res.results[2 * b]["y"].astype(np.float32)
                  + res.results[2 * b + 1]["y"].astype(np.float32) + b_proj)
    return out


# revision 4
# speedup vs baseline: 1.0106x; 1.0106x over previous
"""Multi-head attention (B=4,N=2048,C=768,H=12) on 8 trn2 NeuronCores.

Sharding: core = (batch b, head-half p): 6 heads of one batch per core;
host sums the two half-partials per batch and adds the bias.

Structure (vs the 281us v1 baseline; now ~236us):
 - AV matmul flipped: out[q,d] = est[kv,q].T @ v[kv,d+1] with est as the
   stationary operand -> out free = 65 instead of 512, so attention@V
   costs 42us of PE instead of 83us. The appended ones column makes the
   softmax denominator fall out of column 64.
 - Normalize is a per-partition tensor_scalar on the [q,d] tile (the
   denominator is a per-q scalar); a PE transpose ([q,d]->[d,q], 128
   cycles, odd head straight into PSUM partitions 64-127 via
   tile_position) rebuilds the proj layout.
 - proj accumulates all 3 head-pairs on-device; y ships as bf16.
 - x/wqkv stream as bf16 (cost model charges the moving operand; f32r
   needs >=256-wide outputs to avoid a 4x penalty) and the input fill is
   7 wide DMAs (the sim serializes a ~625ns HWDGE stage per DMA and all
   transfers share one DMA_ENGINES device).
 - Softmax exp mostly on ACT; a per-strip subset of kv tiles uses a
   Schraudolph bit-trick exp on DVE (bf16(int16(s*A+B)), rms 1.8%,
   zero-mean) to keep ACT off the critical path.
 - Flat software-pipelined slot loop over (pair, strip, kv-tile): slot i
   emits AV for seq[i-3], one deferred normalize/transpose/proj item,
   scores+exp for seq[i+3], then deadline/budget-paced qkproj/v-proj
   fillers, so PE (the pacing engine; idle gaps also downclock it) never
   queues an unresolved wait.

PSUM (8 banks): sc 2x[128,2,512]f32 (4) + av [128,2,4,128]f32 (2,
single buf, 512B slot stride so no accumulation group crosses a 2KB
bank; start=True only on each bank's first write since it marks the
whole bank pending-zero) + tp [128,128]bf16 (1) + scratch [128,512]f32
(1, shared by warmup/qkproj/v/proj via one tag).
"""

import sys
from collections import deque

import numpy as np
import ml_dtypes

_REPO = "/opt/trn_rl_repo"
if _REPO not in sys.path:
    sys.path.insert(0, _REPO)

import concourse.bacc as bacc
import concourse.mybir as mybir
import concourse.tile as tile
from concourse.bass_utils import run_bass_kernel_spmd
from concourse.masks import make_identity

B, N, C, H, D = 4, 2048, 768, 12, 64
HL = H // 2          # heads per core
NP = HL // 2         # head pairs per core (3)
SCALE = D ** -0.5
NCORES = 8
KT_C = C // 128      # 6 contraction tiles over C
KVT = N // 128       # 16 kv tiles
NS = N // 512        # 4 query strips of 512

F32 = mybir.dt.float32
F32R = mybir.dt.float32r
BF16 = mybir.dt.bfloat16
I16 = mybir.dt.int16
EXP = mybir.ActivationFunctionType.Exp
MULT = mybir.AluOpType.mult
ADD = mybir.AluOpType.add

# Schraudolph bit-trick exp on DVE: bf16(int16(s*A + B)) ~ exp(s*SCALE).
# rms rel err 1.8%, near-zero mean (c=-7.5); used on a few kv tiles per
# strip to keep ACT off the critical path (PE must stay saturated: any
# PE idle gap drops its clock to 1.2GHz for the next 3us).
SCHRAU_A = 128 * np.log2(np.e) * SCALE
SCHRAU_B = 127.0 * 128 - 7.5
# kv tiles whose exp runs on DVE instead of ACT, by (pair, strip):
# pair0 is PE-overloaded (v-proj + deadline qk chunks), so no offload
# there; pair1/pair2 strips offload 2-3 tiles to keep ACT off the
# critical path while PE stays saturated
SCHRAU_PLAN = {0: ((8,),) * 4,
               1: ((0, 2, 5, 8, 11, 14),) * 4,
               2: ((0, 2, 5, 8, 11, 14),) * 4}


def _schrau_kts(pr, s):
    return SCHRAU_PLAN[pr][s]

_CACHE = {}


def _build():
    nc = bacc.Bacc("TRN2", target_bir_lowering=False, debug=False,
                   num_devices=NCORES)
    xT = nc.dram_tensor("xT", [C, N], BF16, kind="ExternalInput").ap()
    wqkT = nc.dram_tensor("wqkT", [C, 2 * HL * D], BF16, kind="ExternalInput").ap()
    wvT = nc.dram_tensor("wvT", [C, HL * D], BF16, kind="ExternalInput").ap()
    wpT = nc.dram_tensor("wpT", [HL * D, C], BF16, kind="ExternalInput").ap()
    y = nc.dram_tensor("y", [N, C], BF16, kind="ExternalOutput").ap()

    with tile.TileContext(nc) as tc:
        with (
            tc.tile_pool(name="singles", bufs=1) as singles,
            tc.tile_pool(name="sc_p", bufs=2, space="PSUM") as sc_p,
            tc.tile_pool(name="av_p", bufs=1, space="PSUM") as av_p,
            tc.tile_pool(name="tp_p", bufs=1, space="PSUM") as tp_p,
            tc.tile_pool(name="scr_p", bufs=1, space="PSUM") as scr_p,
            tc.tile_pool(name="est_p", bufs=4) as est_p,
            tc.tile_pool(name="stg_p", bufs=6) as stg_p,
            tc.tile_pool(name="rec_p", bufs=2) as rec_p,
            tc.tile_pool(name="ysb_p", bufs=3) as ysb_p,
        ):
            xT_sb = singles.tile([128, KT_C, N], BF16)
            wqk_sb = singles.tile([128, KT_C, 2 * HL * D], BF16)
            wv_sb = singles.tile([128, KT_C, HL * D], BF16)
            wp_sb = singles.tile([128, NP, C], BF16)
            qk_sb = singles.tile([128, 2 * NP, N], F32R)
            v_sb = singles.tile([128, KVT, HL, D + 1], BF16)
            attn_sb = singles.tile([128, NP, N], BF16)
            ident = singles.tile([128, 128], BF16)
            warm_sb = singles.tile([128, 640], BF16)

            # The sim serializes a ~625ns HWDGE stage per DMA instruction,
            # so batch the fill into 7 wide DMAs. xT goes chunk-major on
            # SP's queue (first score work unblocks after ~3.5us); weights
            # go on ACT's queue.
            xT_r = xT.rearrange("(k p) n -> p k n", k=KT_C)
            wqk_r = wqkT.rearrange("(k p) n -> p k n", k=KT_C)
            nc.sync.dma_start(wqk_sb[:, :, HL * D:], wqk_r[:, :, HL * D:])
            nc.sync.dma_start(xT_sb[:, :, 0:512], xT_r[:, :, 0:512])
            nc.sync.dma_start(wqk_sb[:, :, 0:HL * D], wqk_r[:, :, 0:HL * D])
            nc.sync.dma_start(wv_sb, wvT.rearrange("(k p) n -> p k n", k=KT_C))
            for c in range(1, NS):
                csl = slice(c * 512, (c + 1) * 512)
                nc.sync.dma_start(xT_sb[:, :, csl], xT_r[:, :, csl])
            nc.sync.dma_start(wp_sb, wpT.rearrange("(k p) n -> p k n", k=NP))
            nc.vector.memset(warm_sb, 0.0)
            nc.vector.memset(v_sb[:, :, :, D:D + 1], 1.0)
            make_identity(nc, ident)

            # warm the ACT exp table (hardware-only cost; sim ignores it)
            warm_in = rec_p.tile([1, 2], F32, tag="warm")
            warm_out = rec_p.tile([1, 2], BF16, tag="warmo")
            nc.vector.memset(warm_in, 0.0)
            nc.scalar.activation(warm_out, warm_in, EXP, scale=SCALE)

            # PE p-state ramp warmers while the xT DMA fill lands
            for _ in range(10):
                warm_ps = scr_p.tile([128, 512], F32, tag="scr")
                nc.tensor.matmul(warm_ps, lhsT=warm_sb[:, 0:128],
                                 rhs=warm_sb[:, 128:640])

            # ---- filler emission (qkproj / v-proj paced into attention) ----
            def emit_qk_chunk(t, c, via_act=False):
                """qk tile t (0-2: q pairs, 3-5: k pairs), 512-col chunk c.
                via_act stages the PSUM->SBUF copy on the scalar engine,
                which has slack in the exp-offloaded pairs, keeping DVE's
                queue short (the offloaded exp sits in the score-ring WAR
                chain, so DVE latency there stalls PE)."""
                ps = scr_p.tile([128, 512], F32, tag="scr")
                csl = slice(c * 512, (c + 1) * 512)
                for kt in range(KT_C):
                    nc.tensor.matmul(
                        ps,
                        lhsT=wqk_sb[:, kt, t * 128:(t + 1) * 128],
                        rhs=xT_sb[:, kt, csl],
                        start=(kt == 0), stop=(kt == KT_C - 1),
                    )
                if via_act:
                    nc.scalar.copy(qk_sb[:, t, csl], ps)
                else:
                    nc.vector.tensor_copy(qk_sb[:, t, csl], ps)

            def emit_v_mt(pr, mt):
                """v rows for kv tile mt, all 6 heads (pr unused; f32r
                needs a >=256-wide moving operand to stream 1 cycle/row)."""
                ps = scr_p.tile([128, 512], F32, tag="scr")
                for kt in range(KT_C):
                    nc.tensor.matmul(
                        ps[:, 0:384],
                        lhsT=xT_sb[:, kt, mt * 128:(mt + 1) * 128],
                        rhs=wv_sb[:, kt, :],
                        start=(kt == 0), stop=(kt == KT_C - 1),
                    )
                nc.vector.tensor_copy(
                    v_sb[:, mt, :, 0:D],
                    ps[:, 0:384].rearrange("p (h d) -> p h d", h=HL),
                )

            # fillers carry a deadline slot: q-tile chunk c of pair pr is
            # read starting at slot 16*(4*pr+c)-2; k-tile chunk c at slot
            # 64*pr+4*c-2. Budget pacing pulls them earlier when PE has
            # slack; the deadline forces emission when it hasn't.
            QK_CYC, V_CYC = KT_C * 512 + 533, KT_C * 384 + 533
            items = []
            for c in (1, 2, 3):
                # k chunks of pair0, consumed by sc(0,0,4c) at slot 4c-2;
                # paced to match the chunk-major xT DMA arrivals
                items.append((4 * c - 3, QK_CYC, emit_qk_chunk, (3, c)))
                items.append((16 * c - 4, QK_CYC, emit_qk_chunk, (0, c)))
            for c in range(NS):
                items.append((58 + 16 * c, QK_CYC, emit_qk_chunk, (1, c)))
                items.append((52 + 4 * c, QK_CYC, emit_qk_chunk, (4, c)))
                items.append((100 + 4 * c, QK_CYC, emit_qk_chunk, (2, c)))
                items.append((96 + 2 * c, QK_CYC, emit_qk_chunk, (5, c)))
            items.sort(key=lambda it: it[0])
            fillers = deque(items)

            # deferred work carried across strips: transposes of the
            # previous strip (list of fns), proj of the previous strip
            pending = deque()

            def emit_transpose(stg01, pr, s, q2):
                """Transpose both heads' [q,64] tiles into one [128,128]
                psum tile (h1 straight to partitions 64-127), then copy."""
                tp = tp_p.tile([128, 128], BF16, tag="tp")
                for h in range(2):
                    out = tp[64 * h:64 * h + 64, :]
                    nc.tensor.matmul(out, lhsT=stg01[h], rhs=ident,
                                     is_transpose=True,
                                     tile_position=(0, 64 * h))
                csl = slice(s * 512 + q2 * 128, s * 512 + (q2 + 1) * 128)
                nc.vector.tensor_copy(attn_sb[:, pr, csl], tp)

            proj_ysb = {}
            ysb_tail = singles.tile([128, 4, 2, 384], BF16, name="ysb_tail")

            def emit_proj_half(mt, ns_, tail=False):
                if ns_ == 0 and not tail:
                    ysb_new = ysb_p.tile([128, 2, 384], BF16, tag="ysb",
                                         name="ysb")
                    proj_ysb[mt] = ysb_new
                ysb = proj_ysb.get(mt)
                if tail:
                    # scores are done by now: borrow the (idle) sc banks so
                    # the final strip's proj halves double-buffer instead of
                    # serializing on the single scr bank; stage via ACT
                    # (also idle) and ship one batched y DMA at the end
                    sct = sc_p.tile([128, 2, 512], F32, tag="sc", name="sc")
                    yp = sct[:, 0, :]
                else:
                    yp = scr_p.tile([128, 512], F32, tag="scr")
                for pr in range(NP):
                    nc.tensor.matmul(
                        yp[:, 0:384],
                        lhsT=attn_sb[:, pr, mt * 128:(mt + 1) * 128],
                        rhs=wp_sb[:, pr, ns_ * 384:(ns_ + 1) * 384],
                        start=(pr == 0), stop=(pr == NP - 1),
                    )
                if tail:
                    mi = mt - (NS - 1) * 4
                    nc.scalar.copy(ysb_tail[:, mi, ns_, :], yp[:, 0:384])
                    if mi == 1 and ns_ == 1:
                        nc.sync.dma_start(
                            y.rearrange("(m p) c -> p m c", m=KVT)[:, 12:14, :],
                            ysb_tail[:, 0:2].rearrange("p m a b -> p m (a b)"))
                    elif mi == 3 and ns_ == 1:
                        nc.sync.dma_start(
                            y.rearrange("(m p) c -> p m c", m=KVT)[:, 14:16, :],
                            ysb_tail[:, 2:4].rearrange("p m a b -> p m (a b)"))
                    return
                nc.vector.tensor_copy(ysb[:, ns_, :], yp[:, 0:384])
                if ns_ == 1:
                    nc.sync.dma_start(y[mt * 128:(mt + 1) * 128, :],
                                      ysb.rearrange("p a b -> p (a b)"))
                    del proj_ysb[mt]

            # ---- prefill: only chunk-0-dependent work, so the first
            # scores can issue as soon as the first 6 xT DMAs land ----
            emit_qk_chunk(3, 0)              # k tile of pair0, kv 0-3
            emit_qk_chunk(0, 0)              # q tile of pair0, strip0
            v0_emitted = 0

            # ---- attention: flat software-pipelined slot loop ----
            # slot i emits: av for seq[i-2] (its est finished a slot ago),
            # one deferred transpose/proj item, paced fillers, then the
            # scores+exp for seq[i+2]. PE thus never queues an unresolved
            # wait: everything it issues became ready >= 1 slot earlier.
            seq = [(pr, s, kt) for pr in range(NP) for s in range(NS)
                   for kt in range(KVT)]
            STRIP_CAP = globals().get('CAP', 4500)
            ests = {}
            avs = {}

            def emit_scores(pr, s, kt):
                sc = sc_p.tile([128, 2, 512], F32, tag="sc", name="sc")
                qsl = slice(s * 512, (s + 1) * 512)
                for h in range(2):
                    p0, p1 = 64 * h, 64 * h + 64
                    nc.tensor.matmul(
                        sc[:, h, :],
                        lhsT=qk_sb[p0:p1, NP + pr, kt * 128:(kt + 1) * 128],
                        rhs=qk_sb[p0:p1, pr, qsl],
                    )
                if kt in _schrau_kts(pr, s):
                    ei = est_p.tile([128, 2, 512], I16, tag="esti",
                                    bufs=5, name="ei")
                    nc.vector.tensor_scalar(
                        ei, sc, float(SCHRAU_A), float(SCHRAU_B),
                        MULT, ADD)
                    ests[(pr, s, kt)] = ei.bitcast(BF16)
                else:
                    est = est_p.tile([128, 2, 512], BF16, tag="est",
                                     bufs=globals().get("EST_BUFS", 8), name="est")
                    nc.scalar.activation(est, sc, EXP, scale=SCALE)
                    ests[(pr, s, kt)] = est

            def emit_av(pr, s, kt):
                est = ests.pop((pr, s, kt))
                if kt == 0:
                    avs[(pr, s)] = av_p.tile([128, 2, 4, 128], F32,
                                             tag="av", name="av")
                av = avs[(pr, s)]
                for h in range(2):
                    for q2 in range(4):
                        # start only on the bank's first write: a start=True
                        # matmul marks its whole 2KB PSUM bank pending-zero,
                        # so sibling slots must accumulate with start=False
                        nc.tensor.matmul(
                            av[:, h, q2, 0:D + 1],
                            lhsT=est[:, h, q2 * 128:(q2 + 1) * 128],
                            rhs=v_sb[:, kt, 2 * pr + h, :],
                            start=(kt == 0 and q2 == 0),
                            stop=(kt == KVT - 1),
                            skip_group_check=True,
                        )

            def finish_strip(pr, s, last=False):
                """Free the av banks with one copy, then normalize from
                SBUF and queue transposes (+ proj for the last pair).
                For the final strip everything is emitted inline, pipelined
                per q-subtile so the drain chain overlaps."""
                av = avs.pop((pr, s))
                avsb = stg_p.tile([128, 2, 4, D + 1], F32, tag="avsb",
                                  bufs=2, name="avsb")
                nc.vector.tensor_copy(avsb, av[:, :, :, 0:D + 1])
                rec = rec_p.tile([128, 2, 4, 1], F32, tag="rec")
                nc.vector.reciprocal(rec, avsb[:, :, :, D:D + 1])
                stga = stg_p.tile([128, 2, 4, D], BF16, tag="stg",
                                  bufs=2, name="stga")

                def norm_tp(q2):
                    for h in range(2):
                        nc.vector.tensor_scalar(
                            stga[:, h, q2, :], avsb[:, h, q2, 0:D],
                            rec[:, h, q2, :], None, MULT)
                    emit_transpose([stga[:, 0, q2, :], stga[:, 1, q2, :]],
                                   pr, s, q2)

                if last:
                    for q2 in range(4):
                        norm_tp(q2)
                    for mt in (s * 4, s * 4 + 1):
                        for ns_ in range(2):
                            emit_proj_half(mt, ns_, tail=True)
                    for mt in (s * 4 + 2, s * 4 + 3):
                        for ns_ in range(2):
                            emit_proj_half(mt, ns_, tail=True)
                    return
                for q2 in range(4):
                    pending.append((norm_tp, (q2,)))
                if pr == NP - 1:
                    for mt in range(s * 4, s * 4 + 4):
                        for ns_ in range(2):
                            pending.append((emit_proj_half, (mt, ns_)))

            LAG = globals().get('AV_LAG', 3)
            for j in range(LAG):
                emit_scores(*seq[j])
            spent = 0
            for i, trip in enumerate(seq):
                pr, s, kt = trip
                if i >= LAG:
                    emit_av(*seq[i - LAG])
                    opr, os_, okt = seq[i - LAG]
                    if okt == KVT - 1:
                        finish_strip(opr, os_)
                if pending:
                    fn, args = pending.popleft()
                    fn(*args)
                if i + LAG < len(seq):
                    emit_scores(*seq[i + LAG])
                # just-in-time v rows, behind the score stream
                if v0_emitted < KVT and i < KVT:
                    emit_v_mt(0, v0_emitted)
                    v0_emitted += 1
                budget = (i + 1) * STRIP_CAP // KVT
                while fillers and (fillers[0][0] <= i or spent < budget):
                    dl, cyc, fn, args = fillers.popleft()
                    fn(*args)
                    spent += cyc
            for j in range(LAG, 0, -1):
                pj = seq[len(seq) - j]
                if pj[2] == KVT - 1:
                    emit_av(*pj)
                else:
                    emit_av(*pj)
            finish_strip(NP - 1, NS - 1, last=True)
            while pending:
                fn, args = pending.popleft()
                if fn is emit_proj_half:
                    fn(*args, tail=True)
                else:
                    fn(*args)
            while fillers:
                dl, cyc, fn, args = fillers.popleft()
                fn(*args)

    nc.compile()
    return nc


def _get_nc():
    if "nc" not in _CACHE:
        _CACHE["nc"] = _build()
    return _CACHE["nc"]


def _prep_inputs(x, w_qkv, w_proj):
    """Per-core input dicts. Core c: batch c//2, head-half c%2."""
    wq, wk, wv = w_qkv[0:C], w_qkv[C:2 * C], w_qkv[2 * C:3 * C]
    in_maps = []
    for core in range(NCORES):
        b, p = divmod(core, 2)
        heads = [p * HL + j for j in range(HL)]
        qk_rows = np.concatenate(
            [wq[h * D:(h + 1) * D] for h in heads]
            + [wk[h * D:(h + 1) * D] for h in heads], axis=0)   # [768, C]
        v_rows = np.concatenate(
            [wv[h * D:(h + 1) * D] for h in heads], axis=0)     # [384, C]
        p_cols = np.concatenate(
            [w_proj[:, h * D:(h + 1) * D] for h in heads], axis=1)  # [C, 384]
        in_maps.append({
            "xT": np.ascontiguousarray(x[b].T).astype(ml_dtypes.bfloat16),
            "wqkT": np.ascontiguousarray(qk_rows.T).astype(ml_dtypes.bfloat16),
            "wvT": np.ascontiguousarray(v_rows.T).astype(ml_dtypes.bfloat16),
            "wpT": np.ascontiguousarray(p_cols.T).astype(ml_dtypes.bfloat16),
        })
    return in_maps


def kernel(x, w_qkv, w_proj, b_proj, _trace=False):
    x = np.asarray(x, dtype=np.float32)
    w_qkv = np.asarray(w_qkv, dtype=np.float32)
    w_proj = np.asarray(w_proj, dtype=np.float32)
    b_proj = np.asarray(b_proj, dtype=np.float32)

    nc = _get_nc()
    in_maps = _prep_inputs(x, w_qkv, w_proj)
    # The first execution on a cold axon device has been observed to
    # return corrupted results (and the v1 baseline saw transient
    # NRT_EXEC_UNIT_UNRECOVERABLE); always discard a warm-up execution
    # and return the second run's output.
    last_exc = None
    for _attempt in range(3):
        try:
            run_bass_kernel_spmd(nc, in_maps, core_ids=list(range(NCORES)))
            res = run_bass_kernel_spmd(nc, in_maps,
                                       core_ids=list(range(NCORES)),
                                       trace=_trace)
            break
        except Exception as e:
            last_exc = e
    else:
        raise last_exc
    _CACHE["last_results"] = res

    out = np.empty((B, N, C), dtype=np.float32)
    for b in range(B):
        out[b] = (res.results[2 * b]["y"].astype(np.float32)
                  + res.results[2 * b + 1]["y"].astype(np.float32) + b_proj)
    return out


# revision 5
# speedup vs baseline: 1.0153x; 1.0046x over previous
"""Multi-head attention (B=4,N=2048,C=768,H=12) on 8 trn2 NeuronCores.

Sharding: core = (batch b, head-half p): 6 heads of one batch per core;
host sums the two half-partials per batch and adds the bias.

Structure (vs the 281us v1 baseline; now ~236us):
 - AV matmul flipped: out[q,d] = est[kv,q].T @ v[kv,d+1] with est as the
   stationary operand -> out free = 65 instead of 512, so attention@V
   costs 42us of PE instead of 83us. The appended ones column makes the
   softmax denominator fall out of column 64.
 - Normalize is a per-partition tensor_scalar on the [q,d] tile (the
   denominator is a per-q scalar); a PE transpose ([q,d]->[d,q], 128
   cycles, odd head straight into PSUM partitions 64-127 via
   tile_position) rebuilds the proj layout.
 - proj accumulates all 3 head-pairs on-device; y ships as bf16.
 - x/wqkv stream as bf16 (cost model charges the moving operand; f32r
   needs >=256-wide outputs to avoid a 4x penalty) and the input fill is
   7 wide DMAs (the sim serializes a ~625ns HWDGE stage per DMA and all
   transfers share one DMA_ENGINES device).
 - Softmax exp mostly on ACT; a per-strip subset of kv tiles uses a
   Schraudolph bit-trick exp on DVE (bf16(int16(s*A+B)), rms 1.8%,
   zero-mean) to keep ACT off the critical path.
 - Flat software-pipelined slot loop over (pair, strip, kv-tile): slot i
   emits AV for seq[i-3], one deferred normalize/transpose/proj item,
   scores+exp for seq[i+3], then deadline/budget-paced qkproj/v-proj
   fillers, so PE (the pacing engine; idle gaps also downclock it) never
   queues an unresolved wait.

PSUM (8 banks): sc 2x[128,2,512]f32 (4) + av [128,2,4,128]f32 (2,
single buf, 512B slot stride so no accumulation group crosses a 2KB
bank; start=True only on each bank's first write since it marks the
whole bank pending-zero) + tp [128,128]bf16 (1) + scratch [128,512]f32
(1, shared by warmup/qkproj/v/proj via one tag).
"""

import sys
from collections import deque

import numpy as np
import ml_dtypes

_REPO = "/opt/trn_rl_repo"
if _REPO not in sys.path:
    sys.path.insert(0, _REPO)

import concourse.bacc as bacc
import concourse.mybir as mybir
import concourse.tile as tile
from concourse.bass_utils import run_bass_kernel_spmd
from concourse.masks import make_identity

B, N, C, H, D = 4, 2048, 768, 12, 64
HL = H // 2          # heads per core
NP = HL // 2         # head pairs per core (3)
SCALE = D ** -0.5
NCORES = 8
KT_C = C // 128      # 6 contraction tiles over C
KVT = N // 128       # 16 kv tiles
NS = N // 512        # 4 query strips of 512

F32 = mybir.dt.float32
F32R = mybir.dt.float32r
BF16 = mybir.dt.bfloat16
I16 = mybir.dt.int16
EXP = mybir.ActivationFunctionType.Exp
MULT = mybir.AluOpType.mult
ADD = mybir.AluOpType.add

# Schraudolph bit-trick exp on DVE: bf16(int16(s*A + B)) ~ exp(s*SCALE).
# rms rel err 1.8%, near-zero mean (c=-7.5); used on a few kv tiles per
# strip to keep ACT off the critical path (PE must stay saturated: any
# PE idle gap drops its clock to 1.2GHz for the next 3us).
SCHRAU_A = 128 * np.log2(np.e) * SCALE
SCHRAU_B = 127.0 * 128 - 7.5
# kv tiles whose exp runs on DVE instead of ACT, by (pair, strip):
# pair0 is PE-overloaded (v-proj + deadline qk chunks), so no offload
# there; pair1/pair2 strips offload 2-3 tiles to keep ACT off the
# critical path while PE stays saturated
SCHRAU_PLAN = {0: ((8,),) * 4,
               1: ((0, 2, 5, 8, 11, 14),) * 4,
               2: ((0, 2, 5, 8, 11, 14),) * 4}


def _schrau_kts(pr, s):
    return SCHRAU_PLAN[pr][s]

_CACHE = {}


def _build():
    nc = bacc.Bacc("TRN2", target_bir_lowering=False, debug=False,
                   num_devices=NCORES)
    xT = nc.dram_tensor("xT", [C, N], BF16, kind="ExternalInput").ap()
    wqkT = nc.dram_tensor("wqkT", [C, 2 * HL * D], BF16, kind="ExternalInput").ap()
    wvT = nc.dram_tensor("wvT", [C, HL * D], BF16, kind="ExternalInput").ap()
    wpT = nc.dram_tensor("wpT", [HL * D, C], BF16, kind="ExternalInput").ap()
    y = nc.dram_tensor("y", [N, C], BF16, kind="ExternalOutput").ap()

    with tile.TileContext(nc) as tc:
        with (
            tc.tile_pool(name="singles", bufs=1) as singles,
            tc.tile_pool(name="sc_p", bufs=3, space="PSUM") as sc_p,
            tc.tile_pool(name="av_p", bufs=1, space="PSUM") as av_p,
            tc.tile_pool(name="est_p", bufs=4) as est_p,
            tc.tile_pool(name="stg_p", bufs=6) as stg_p,
            tc.tile_pool(name="rec_p", bufs=2) as rec_p,
            tc.tile_pool(name="ysb_p", bufs=3) as ysb_p,
        ):
            xT_sb = singles.tile([128, KT_C, N], BF16)
            wqk_sb = singles.tile([128, KT_C, 2 * HL * D], BF16)
            wv_sb = singles.tile([128, KT_C, HL * D], BF16)
            wp_sb = singles.tile([128, NP, C], BF16)
            qk_sb = singles.tile([128, 2 * NP, N], F32R)
            v_sb = singles.tile([128, KVT, HL, D + 1], BF16)
            attn_sb = singles.tile([128, NP, N], BF16)
            ident = singles.tile([128, 128], BF16)
            warm_sb = singles.tile([128, 640], BF16)

            # The sim serializes a ~625ns HWDGE stage per DMA instruction,
            # so batch the fill into 7 wide DMAs. xT goes chunk-major on
            # SP's queue (first score work unblocks after ~3.5us); weights
            # go on ACT's queue.
            xT_r = xT.rearrange("(k p) n -> p k n", k=KT_C)
            wqk_r = wqkT.rearrange("(k p) n -> p k n", k=KT_C)
            nc.sync.dma_start(wqk_sb[:, :, HL * D:], wqk_r[:, :, HL * D:])
            nc.sync.dma_start(xT_sb[:, :, 0:512], xT_r[:, :, 0:512])
            nc.sync.dma_start(wqk_sb[:, :, 0:HL * D], wqk_r[:, :, 0:HL * D])
            nc.sync.dma_start(wv_sb, wvT.rearrange("(k p) n -> p k n", k=KT_C))
            for c in range(1, NS):
                csl = slice(c * 512, (c + 1) * 512)
                nc.sync.dma_start(xT_sb[:, :, csl], xT_r[:, :, csl])
            nc.sync.dma_start(wp_sb, wpT.rearrange("(k p) n -> p k n", k=NP))
            nc.vector.memset(warm_sb, 0.0)
            nc.vector.memset(v_sb[:, :, :, D:D + 1], 1.0)
            make_identity(nc, ident)

            # warm the ACT exp table (hardware-only cost; sim ignores it)
            warm_in = rec_p.tile([1, 2], F32, tag="warm")
            warm_out = rec_p.tile([1, 2], BF16, tag="warmo")
            nc.vector.memset(warm_in, 0.0)
            nc.scalar.activation(warm_out, warm_in, EXP, scale=SCALE)

            def ring_tile():
                t = sc_p.tile([128, 2, 512], F32, tag="sc", name="sc")
                return t

            # PE p-state ramp warmers while the xT DMA fill lands
            for _ in range(10):
                warm_ps = ring_tile()
                nc.tensor.matmul(warm_ps[:, 0, :], lhsT=warm_sb[:, 0:128],
                                 rhs=warm_sb[:, 128:640])

            # ---- filler emission (qkproj / v-proj paced into attention) ----
            def emit_qk_chunk(t, c, via_act=False):
                """qk tile t (0-2: q pairs, 3-5: k pairs), 512-col chunk c.
                via_act stages the PSUM->SBUF copy on the scalar engine,
                which has slack in the exp-offloaded pairs, keeping DVE's
                queue short (the offloaded exp sits in the score-ring WAR
                chain, so DVE latency there stalls PE)."""
                ps = ring_tile()[:, 0, :]
                csl = slice(c * 512, (c + 1) * 512)
                for kt in range(KT_C):
                    nc.tensor.matmul(
                        ps,
                        lhsT=wqk_sb[:, kt, t * 128:(t + 1) * 128],
                        rhs=xT_sb[:, kt, csl],
                        start=(kt == 0), stop=(kt == KT_C - 1),
                    )
                if via_act:
                    nc.scalar.copy(qk_sb[:, t, csl], ps)
                else:
                    nc.vector.tensor_copy(qk_sb[:, t, csl], ps)

            def emit_v_mt(pr, mt):
                """v rows for kv tile mt, all 6 heads (pr unused; f32r
                needs a >=256-wide moving operand to stream 1 cycle/row)."""
                ps = ring_tile()[:, 0, :]
                for kt in range(KT_C):
                    nc.tensor.matmul(
                        ps[:, 0:384],
                        lhsT=xT_sb[:, kt, mt * 128:(mt + 1) * 128],
                        rhs=wv_sb[:, kt, :],
                        start=(kt == 0), stop=(kt == KT_C - 1),
                    )
                nc.vector.tensor_copy(
                    v_sb[:, mt, :, 0:D],
                    ps[:, 0:384].rearrange("p (h d) -> p h d", h=HL),
                )

            # fillers carry a deadline slot: q-tile chunk c of pair pr is
            # read starting at slot 16*(4*pr+c)-2; k-tile chunk c at slot
            # 64*pr+4*c-2. Budget pacing pulls them earlier when PE has
            # slack; the deadline forces emission when it hasn't.
            QK_CYC, V_CYC = KT_C * 512 + 533, KT_C * 384 + 533
            items = []
            for c in (1, 2, 3):
                # k chunks of pair0, consumed by sc(0,0,4c) at slot 4c-2;
                # paced to match the chunk-major xT DMA arrivals
                items.append((4 * c - 3, QK_CYC, emit_qk_chunk, (3, c)))
                items.append((16 * c - 4, QK_CYC, emit_qk_chunk, (0, c)))
            for c in range(NS):
                items.append((58 + 16 * c, QK_CYC, emit_qk_chunk, (1, c)))
                items.append((52 + 4 * c, QK_CYC, emit_qk_chunk, (4, c)))
                items.append((100 + 4 * c, QK_CYC, emit_qk_chunk, (2, c)))
                items.append((96 + 2 * c, QK_CYC, emit_qk_chunk, (5, c)))
            items.sort(key=lambda it: it[0])
            fillers = deque(items)

            # deferred work carried across strips: transposes of the
            # previous strip (list of fns), proj of the previous strip
            pending = deque()

            def emit_transpose(stg01, pr, s, q2):
                """Transpose both heads' [q,64] tiles into one [128,128]
                psum tile (h1 straight to partitions 64-127), then copy."""
                tp = ring_tile()[:, 0, 0:64].bitcast(BF16)
                for h in range(2):
                    out = tp[64 * h:64 * h + 64, :]
                    nc.tensor.matmul(out, lhsT=stg01[h], rhs=ident,
                                     is_transpose=True,
                                     tile_position=(0, 64 * h))
                csl = slice(s * 512 + q2 * 128, s * 512 + (q2 + 1) * 128)
                nc.vector.tensor_copy(attn_sb[:, pr, csl], tp)

            proj_ysb = {}
            ysb_tail = singles.tile([128, 4, 2, 384], BF16, name="ysb_tail")

            def emit_proj_half(mt, ns_, tail=False):
                if ns_ == 0 and not tail:
                    ysb_new = ysb_p.tile([128, 2, 384], BF16, tag="ysb",
                                         name="ysb")
                    proj_ysb[mt] = ysb_new
                ysb = proj_ysb.get(mt)
                yp = ring_tile()[:, 0, :]
                for pr in range(NP):
                    nc.tensor.matmul(
                        yp[:, 0:384],
                        lhsT=attn_sb[:, pr, mt * 128:(mt + 1) * 128],
                        rhs=wp_sb[:, pr, ns_ * 384:(ns_ + 1) * 384],
                        start=(pr == 0), stop=(pr == NP - 1),
                    )
                if tail:
                    mi = mt - (NS - 1) * 4
                    nc.scalar.copy(ysb_tail[:, mi, ns_, :], yp[:, 0:384])
                    if mi == 1 and ns_ == 1:
                        nc.sync.dma_start(
                            y.rearrange("(m p) c -> p m c", m=KVT)[:, 12:14, :],
                            ysb_tail[:, 0:2].rearrange("p m a b -> p m (a b)"))
                    elif mi == 3 and ns_ == 1:
                        nc.sync.dma_start(
                            y.rearrange("(m p) c -> p m c", m=KVT)[:, 14:16, :],
                            ysb_tail[:, 2:4].rearrange("p m a b -> p m (a b)"))
                    return
                nc.vector.tensor_copy(ysb[:, ns_, :], yp[:, 0:384])
                if ns_ == 1:
                    nc.sync.dma_start(y[mt * 128:(mt + 1) * 128, :],
                                      ysb.rearrange("p a b -> p (a b)"))
                    del proj_ysb[mt]

            # ---- prefill: only chunk-0-dependent work, so the first
            # scores can issue as soon as the first 6 xT DMAs land ----
            emit_qk_chunk(3, 0)              # k tile of pair0, kv 0-3
            emit_qk_chunk(0, 0)              # q tile of pair0, strip0
            v0_emitted = 0

            # ---- attention: flat software-pipelined slot loop ----
            # slot i emits: av for seq[i-2] (its est finished a slot ago),
            # one deferred transpose/proj item, paced fillers, then the
            # scores+exp for seq[i+2]. PE thus never queues an unresolved
            # wait: everything it issues became ready >= 1 slot earlier.
            seq = [(pr, s, kt) for pr in range(NP) for s in range(NS)
                   for kt in range(KVT)]
            STRIP_CAP = globals().get('CAP', 4500)
            ests = {}
            avs = {}

            def emit_scores(pr, s, kt):
                sc = ring_tile()
                qsl = slice(s * 512, (s + 1) * 512)
                for h in range(2):
                    p0, p1 = 64 * h, 64 * h + 64
                    nc.tensor.matmul(
                        sc[:, h, :],
                        lhsT=qk_sb[p0:p1, NP + pr, kt * 128:(kt + 1) * 128],
                        rhs=qk_sb[p0:p1, pr, qsl],
                    )
                if kt in _schrau_kts(pr, s):
                    ei = est_p.tile([128, 2, 512], I16, tag="esti",
                                    bufs=5, name="ei")
                    nc.vector.tensor_scalar(
                        ei, sc, float(SCHRAU_A), float(SCHRAU_B),
                        MULT, ADD)
                    ests[(pr, s, kt)] = ei.bitcast(BF16)
                else:
                    est = est_p.tile([128, 2, 512], BF16, tag="est",
                                     bufs=globals().get("EST_BUFS", 8), name="est")
                    nc.scalar.activation(est, sc, EXP, scale=SCALE)
                    ests[(pr, s, kt)] = est

            def emit_av(pr, s, kt):
                est = ests.pop((pr, s, kt))
                if kt == 0:
                    avs[(pr, s)] = av_p.tile([128, 2, 4, 128], F32,
                                             tag="av", name="av")
                av = avs[(pr, s)]
                for h in range(2):
                    for q2 in range(4):
                        # start only on the bank's first write: a start=True
                        # matmul marks its whole 2KB PSUM bank pending-zero,
                        # so sibling slots must accumulate with start=False
                        nc.tensor.matmul(
                            av[:, h, q2, 0:D + 1],
                            lhsT=est[:, h, q2 * 128:(q2 + 1) * 128],
                            rhs=v_sb[:, kt, 2 * pr + h, :],
                            start=(kt == 0 and q2 == 0),
                            stop=(kt == KVT - 1),
                            skip_group_check=True,
                        )

            def finish_strip(pr, s, last=False):
                """Free the av banks with one copy, then normalize from
                SBUF and queue transposes (+ proj for the last pair).
                For the final strip everything is emitted inline, pipelined
                per q-subtile so the drain chain overlaps."""
                av = avs.pop((pr, s))
                avsb = stg_p.tile([128, 2, 4, D + 1], F32, tag="avsb",
                                  bufs=2, name="avsb")
                nc.vector.tensor_copy(avsb, av[:, :, :, 0:D + 1])
                rec = rec_p.tile([128, 2, 4, 1], F32, tag="rec")
                nc.vector.reciprocal(rec, avsb[:, :, :, D:D + 1])
                stga = stg_p.tile([128, 2, 4, D], BF16, tag="stg",
                                  bufs=2, name="stga")

                def norm_tp(q2):
                    for h in range(2):
                        nc.vector.tensor_scalar(
                            stga[:, h, q2, :], avsb[:, h, q2, 0:D],
                            rec[:, h, q2, :], None, MULT)
                    emit_transpose([stga[:, 0, q2, :], stga[:, 1, q2, :]],
                                   pr, s, q2)

                if last:
                    for q2 in range(4):
                        norm_tp(q2)
                    for mt in (s * 4, s * 4 + 1):
                        for ns_ in range(2):
                            emit_proj_half(mt, ns_, tail=True)
                    for mt in (s * 4 + 2, s * 4 + 3):
                        for ns_ in range(2):
                            emit_proj_half(mt, ns_, tail=True)
                    return
                for q2 in range(4):
                    pending.append((norm_tp, (q2,)))
                if pr == NP - 1:
                    for mt in range(s * 4, s * 4 + 4):
                        for ns_ in range(2):
                            pending.append((emit_proj_half, (mt, ns_)))

            LAG = globals().get('AV_LAG', 3)
            for j in range(LAG):
                emit_scores(*seq[j])
            spent = 0
            for i, trip in enumerate(seq):
                pr, s, kt = trip
                if i >= LAG:
                    emit_av(*seq[i - LAG])
                    opr, os_, okt = seq[i - LAG]
                    if okt == KVT - 1:
                        finish_strip(opr, os_)
                if pending:
                    fn, args = pending.popleft()
                    fn(*args)
                if i + LAG < len(seq):
                    emit_scores(*seq[i + LAG])
                # just-in-time v rows, behind the score stream
                if v0_emitted < KVT and i < KVT:
                    emit_v_mt(0, v0_emitted)
                    v0_emitted += 1
                budget = (i + 1) * STRIP_CAP // KVT
                while fillers and (fillers[0][0] <= i or spent < budget):
                    dl, cyc, fn, args = fillers.popleft()
                    fn(*args)
                    spent += cyc
            for j in range(LAG, 0, -1):
                pj = seq[len(seq) - j]
                if pj[2] == KVT - 1:
                    emit_av(*pj)
                else:
                    emit_av(*pj)
            finish_strip(NP - 1, NS - 1, last=True)
            while pending:
                fn, args = pending.popleft()
                if fn is emit_proj_half:
                    fn(*args, tail=True)
                else:
                    fn(*args)
            while fillers:
                dl, cyc, fn, args = fillers.popleft()
                fn(*args)

    nc.compile()
    return nc


def _get_nc():
    if "nc" not in _CACHE:
        _CACHE["nc"] = _build()
    return _CACHE["nc"]


def _prep_inputs(x, w_qkv, w_proj):
    """Per-core input dicts. Core c: batch c//2, head-half c%2."""
    wq, wk, wv = w_qkv[0:C], w_qkv[C:2 * C], w_qkv[2 * C:3 * C]
    in_maps = []
    for core in range(NCORES):
        b, p = divmod(core, 2)
        heads = [p * HL + j for j in range(HL)]
        qk_rows = np.concatenate(
            [wq[h * D:(h + 1) * D] for h in heads]
            + [wk[h * D:(h + 1) * D] for h in heads], axis=0)   # [768, C]
        v_rows = np.concatenate(
            [wv[h * D:(h + 1) * D] for h in heads], axis=0)     # [384, C]
        p_cols = np.concatenate(
            [w_proj[:, h * D:(h + 1) * D] for h in heads], axis=1)  # [C, 384]
        in_maps.append({
            "xT": np.ascontiguousarray(x[b].T).astype(ml_dtypes.bfloat16),
            "wqkT": np.ascontiguousarray(qk_rows.T).astype(ml_dtypes.bfloat16),
            "wvT": np.ascontiguousarray(v_rows.T).astype(ml_dtypes.bfloat16),
            "wpT": np.ascontiguousarray(p_cols.T).astype(ml_dtypes.bfloat16),
        })
    return in_maps


def kernel(x, w_qkv, w_proj, b_proj, _trace=False):
    x = np.asarray(x, dtype=np.float32)
    w_qkv = np.asarray(w_qkv, dtype=np.float32)
    w_proj = np.asarray(w_proj, dtype=np.float32)
    b_proj = np.asarray(b_proj, dtype=np.float32)

    nc = _get_nc()
    in_maps = _prep_inputs(x, w_qkv, w_proj)
    # The first execution on a cold axon device has been observed to
    # return corrupted results (and the v1 baseline saw transient
    # NRT_EXEC_UNIT_UNRECOVERABLE); always discard a warm-up execution
    # and return the second run's output.
    last_exc = None
    for _attempt in range(3):
        try:
            run_bass_kernel_spmd(nc, in_maps, core_ids=list(range(NCORES)))
            res = run_bass_kernel_spmd(nc, in_maps,
                                       core_ids=list(range(NCORES)),
                                       trace=_trace)
            break
        except Exception as e:
            last_exc = e
    else:
        raise last_exc
    _CACHE["last_results"] = res

    out = np.empty((B, N, C), dtype=np.float32)
    for b in range(B):
        out[b] = (res.results[2 * b]["y"].astype(np.float32)
                  + res.results[2 * b + 1]["y"].astype(np.float32) + b_proj)
    return out


# revision 6
# speedup vs baseline: 1.0299x; 1.0144x over previous
"""Multi-head attention (B=4,N=2048,C=768,H=12) on 8 trn2 NeuronCores.

Sharding: core = (batch b, head-half p): 6 heads of one batch per core;
host sums the two half-partials per batch and adds the bias.

Structure (vs the 281us v1 baseline; now ~236us):
 - AV matmul flipped: out[q,d] = est[kv,q].T @ v[kv,d+1] with est as the
   stationary operand -> out free = 65 instead of 512, so attention@V
   costs 42us of PE instead of 83us. The appended ones column makes the
   softmax denominator fall out of column 64.
 - Normalize is a per-partition tensor_scalar on the [q,d] tile (the
   denominator is a per-q scalar); a PE transpose ([q,d]->[d,q], 128
   cycles, odd head straight into PSUM partitions 64-127 via
   tile_position) rebuilds the proj layout.
 - proj accumulates all 3 head-pairs on-device; y ships as bf16.
 - x/wqkv stream as bf16 (cost model charges the moving operand; f32r
   needs >=256-wide outputs to avoid a 4x penalty) and the input fill is
   7 wide DMAs (the sim serializes a ~625ns HWDGE stage per DMA and all
   transfers share one DMA_ENGINES device).
 - Softmax exp mostly on ACT; a per-strip subset of kv tiles uses a
   Schraudolph bit-trick exp on DVE (bf16(int16(s*A+B)), rms 1.8%,
   zero-mean) to keep ACT off the critical path.
 - Flat software-pipelined slot loop over (pair, strip, kv-tile): slot i
   emits AV for seq[i-3], one deferred normalize/transpose/proj item,
   scores+exp for seq[i+3], then deadline/budget-paced qkproj/v-proj
   fillers, so PE (the pacing engine; idle gaps also downclock it) never
   queues an unresolved wait.

PSUM (8 banks): sc 2x[128,2,512]f32 (4) + av [128,2,4,128]f32 (2,
single buf, 512B slot stride so no accumulation group crosses a 2KB
bank; start=True only on each bank's first write since it marks the
whole bank pending-zero) + tp [128,128]bf16 (1) + scratch [128,512]f32
(1, shared by warmup/qkproj/v/proj via one tag).
"""

import sys
from collections import deque

import numpy as np
import ml_dtypes

_REPO = "/opt/trn_rl_repo"
if _REPO not in sys.path:
    sys.path.insert(0, _REPO)

import concourse.bacc as bacc
import concourse.mybir as mybir
import concourse.tile as tile
from concourse.bass_utils import run_bass_kernel_spmd
from concourse.masks import make_identity

B, N, C, H, D = 4, 2048, 768, 12, 64
HL = H // 2          # heads per core
NP = HL // 2         # head pairs per core (3)
SCALE = D ** -0.5
NCORES = 8
KT_C = C // 128      # 6 contraction tiles over C
KVT = N // 128       # 16 kv tiles
NS = N // 512        # 4 query strips of 512

F32 = mybir.dt.float32
F32R = mybir.dt.float32r
BF16 = mybir.dt.bfloat16
I16 = mybir.dt.int16
EXP = mybir.ActivationFunctionType.Exp
MULT = mybir.AluOpType.mult
ADD = mybir.AluOpType.add

# Schraudolph bit-trick exp on DVE: bf16(int16(s*A + B)) ~ exp(s*SCALE).
# rms rel err 1.8%, near-zero mean (c=-7.5); used on a few kv tiles per
# strip to keep ACT off the critical path (PE must stay saturated: any
# PE idle gap drops its clock to 1.2GHz for the next 3us).
SCHRAU_A = 128 * np.log2(np.e) * SCALE
SCHRAU_B = 127.0 * 128 - 7.5
# kv tiles whose exp runs on DVE instead of ACT, by (pair, strip):
# pair0 is PE-overloaded (v-proj + deadline qk chunks), so no offload
# there; pair1/pair2 strips offload 2-3 tiles to keep ACT off the
# critical path while PE stays saturated
SCHRAU_PLAN = {0: ((8,),) * 4,
               1: ((0, 2, 5, 8, 11, 14),) * 4,
               2: ((0, 2, 5, 8, 11, 14),) * 4}


def _schrau_kts(pr, s):
    return SCHRAU_PLAN[pr][s]

_CACHE = {}


def _build():
    nc = bacc.Bacc("TRN2", target_bir_lowering=False, debug=False,
                   num_devices=NCORES)
    xT = nc.dram_tensor("xT", [C, N], BF16, kind="ExternalInput").ap()
    wqkT = nc.dram_tensor("wqkT", [C, 2 * HL * D], BF16, kind="ExternalInput").ap()
    wvT = nc.dram_tensor("wvT", [C, HL * D], BF16, kind="ExternalInput").ap()
    wpT = nc.dram_tensor("wpT", [HL * D, C], BF16, kind="ExternalInput").ap()
    y = nc.dram_tensor("y", [N, C], BF16, kind="ExternalOutput").ap()

    with tile.TileContext(nc) as tc:
        with (
            tc.tile_pool(name="singles", bufs=1) as singles,
            tc.tile_pool(name="sc_p", bufs=3, space="PSUM") as sc_p,
            tc.tile_pool(name="av_p", bufs=1, space="PSUM") as av_p,
            tc.tile_pool(name="est_p", bufs=4) as est_p,
            tc.tile_pool(name="stg_p", bufs=6) as stg_p,
            tc.tile_pool(name="rec_p", bufs=2) as rec_p,
            tc.tile_pool(name="ysb_p", bufs=3) as ysb_p,
        ):
            xT_sb = singles.tile([128, KT_C, N], BF16)
            wqk_sb = singles.tile([128, KT_C, 2 * HL * D], BF16)
            wv_sb = singles.tile([128, KT_C, HL * D], BF16)
            wp_sb = singles.tile([128, NP, C], BF16)
            qk_sb = singles.tile([128, 2 * NP, N], F32R)
            v_sb = singles.tile([128, KVT, HL, D + 1], BF16)
            attn_sb = singles.tile([128, NP, N], BF16)
            ident = singles.tile([128, 128], BF16)
            warm_sb = singles.tile([128, 640], BF16)

            # The sim serializes a ~625ns HWDGE stage per DMA instruction,
            # so batch the fill into 7 wide DMAs. xT goes chunk-major on
            # SP's queue (first score work unblocks after ~3.5us); weights
            # go on ACT's queue.
            xT_r = xT.rearrange("(k p) n -> p k n", k=KT_C)
            wqk_r = wqkT.rearrange("(k p) n -> p k n", k=KT_C)
            nc.sync.dma_start(wqk_sb[:, :, HL * D:], wqk_r[:, :, HL * D:])
            nc.sync.dma_start(xT_sb[:, :, 0:512], xT_r[:, :, 0:512])
            nc.sync.dma_start(wqk_sb[:, :, 0:HL * D], wqk_r[:, :, 0:HL * D])
            nc.sync.dma_start(wv_sb, wvT.rearrange("(k p) n -> p k n", k=KT_C))
            for c in range(1, NS):
                csl = slice(c * 512, (c + 1) * 512)
                nc.sync.dma_start(xT_sb[:, :, csl], xT_r[:, :, csl])
            nc.sync.dma_start(wp_sb, wpT.rearrange("(k p) n -> p k n", k=NP))
            nc.vector.memset(warm_sb, 0.0)
            nc.vector.memset(v_sb[:, :, :, D:D + 1], 1.0)
            make_identity(nc, ident)

            # warm the ACT exp table (hardware-only cost; sim ignores it)
            warm_in = rec_p.tile([1, 2], F32, tag="warm")
            warm_out = rec_p.tile([1, 2], BF16, tag="warmo")
            nc.vector.memset(warm_in, 0.0)
            nc.scalar.activation(warm_out, warm_in, EXP, scale=SCALE)

            def ring_tile():
                t = sc_p.tile([128, 2, 512], F32, tag="sc", name="sc")
                return t

            # PE p-state ramp warmers while the xT DMA fill lands
            for _ in range(10):
                warm_ps = ring_tile()
                nc.tensor.matmul(warm_ps[:, 0, :], lhsT=warm_sb[:, 0:128],
                                 rhs=warm_sb[:, 128:640])

            # ---- filler emission (qkproj / v-proj paced into attention) ----
            def emit_qk_chunk(t, c, via_act=False):
                """qk tile t (0-2: q pairs, 3-5: k pairs), 512-col chunk c.
                via_act stages the PSUM->SBUF copy on the scalar engine,
                which has slack in the exp-offloaded pairs, keeping DVE's
                queue short (the offloaded exp sits in the score-ring WAR
                chain, so DVE latency there stalls PE)."""
                ps = ring_tile()[:, 0, :]
                csl = slice(c * 512, (c + 1) * 512)
                for kt in range(KT_C):
                    nc.tensor.matmul(
                        ps,
                        lhsT=wqk_sb[:, kt, t * 128:(t + 1) * 128],
                        rhs=xT_sb[:, kt, csl],
                        start=(kt == 0), stop=(kt == KT_C - 1),
                    )
                if via_act:
                    nc.scalar.copy(qk_sb[:, t, csl], ps)
                else:
                    nc.vector.tensor_copy(qk_sb[:, t, csl], ps)

            def emit_v_mt(pr, mt):
                """v rows for kv tile mt, all 6 heads (pr unused; f32r
                needs a >=256-wide moving operand to stream 1 cycle/row)."""
                ps = ring_tile()[:, 0, :]
                for kt in range(KT_C):
                    nc.tensor.matmul(
                        ps[:, 0:384],
                        lhsT=xT_sb[:, kt, mt * 128:(mt + 1) * 128],
                        rhs=wv_sb[:, kt, :],
                        start=(kt == 0), stop=(kt == KT_C - 1),
                    )
                nc.vector.tensor_copy(
                    v_sb[:, mt, :, 0:D],
                    ps[:, 0:384].rearrange("p (h d) -> p h d", h=HL),
                )

            # fillers carry a deadline slot: q-tile chunk c of pair pr is
            # read starting at slot 16*(4*pr+c)-2; k-tile chunk c at slot
            # 64*pr+4*c-2. Budget pacing pulls them earlier when PE has
            # slack; the deadline forces emission when it hasn't.
            QK_CYC, V_CYC = KT_C * 512 + 533, KT_C * 384 + 533
            items = []
            for c in (1, 2, 3):
                # k chunks of pair0, consumed by sc(0,0,4c) at slot 4c-2;
                # paced to match the chunk-major xT DMA arrivals
                items.append((4 * c - 3, QK_CYC, emit_qk_chunk, (3, c)))
                items.append((16 * c - 4, QK_CYC, emit_qk_chunk, (0, c)))
            for c in range(NS):
                items.append((58 + 16 * c, QK_CYC, emit_qk_chunk, (1, c)))
                items.append((52 + 4 * c, QK_CYC, emit_qk_chunk, (4, c)))
                items.append((100 + 4 * c, QK_CYC, emit_qk_chunk, (2, c)))
                items.append((96 + 2 * c, QK_CYC, emit_qk_chunk, (5, c)))
            items.sort(key=lambda it: it[0])
            fillers = deque(items)

            # deferred work carried across strips: transposes of the
            # previous strip (list of fns), proj of the previous strip
            pending = deque()

            def emit_transpose(stg01, pr, s, q2):
                """Transpose both heads' [q,64] tiles into one [128,128]
                psum tile (h1 straight to partitions 64-127), then copy."""
                tp = ring_tile()[:, 0, 0:64].bitcast(BF16)
                for h in range(2):
                    out = tp[64 * h:64 * h + 64, :]
                    nc.tensor.matmul(out, lhsT=stg01[h], rhs=ident,
                                     is_transpose=True,
                                     tile_position=(0, 64 * h))
                csl = slice(s * 512 + q2 * 128, s * 512 + (q2 + 1) * 128)
                nc.vector.tensor_copy(attn_sb[:, pr, csl], tp)

            proj_ysb = {}
            ysb_tail = singles.tile([128, 4, 2, 384], BF16, name="ysb_tail")

            def emit_proj_half(mt, ns_, tail=False):
                if ns_ == 0 and not tail:
                    ysb_new = ysb_p.tile([128, 2, 384], BF16, tag="ysb",
                                         name="ysb")
                    proj_ysb[mt] = ysb_new
                ysb = proj_ysb.get(mt)
                yp = ring_tile()[:, 0, :]
                for pr in range(NP):
                    nc.tensor.matmul(
                        yp[:, 0:384],
                        lhsT=attn_sb[:, pr, mt * 128:(mt + 1) * 128],
                        rhs=wp_sb[:, pr, ns_ * 384:(ns_ + 1) * 384],
                        start=(pr == 0), stop=(pr == NP - 1),
                    )
                if tail:
                    mi = mt - (NS - 1) * 4
                    nc.scalar.copy(ysb_tail[:, mi, ns_, :], yp[:, 0:384])
                    if mi == 1 and ns_ == 1:
                        nc.sync.dma_start(
                            y.rearrange("(m p) c -> p m c", m=KVT)[:, 12:14, :],
                            ysb_tail[:, 0:2].rearrange("p m a b -> p m (a b)"))
                    elif mi == 3 and ns_ == 1:
                        nc.sync.dma_start(
                            y.rearrange("(m p) c -> p m c", m=KVT)[:, 14:16, :],
                            ysb_tail[:, 2:4].rearrange("p m a b -> p m (a b)"))
                    return
                nc.vector.tensor_copy(ysb[:, ns_, :], yp[:, 0:384])
                if ns_ == 1:
                    nc.sync.dma_start(y[mt * 128:(mt + 1) * 128, :],
                                      ysb.rearrange("p a b -> p (a b)"))
                    del proj_ysb[mt]

            # ---- prefill: only chunk-0-dependent work, so the first
            # scores can issue as soon as the first 6 xT DMAs land ----
            emit_qk_chunk(3, 0)              # k tile of pair0, kv 0-3
            emit_qk_chunk(0, 0)              # q tile of pair0, strip0
            v0_emitted = 0

            # ---- attention: flat software-pipelined slot loop ----
            # slot i emits: av for seq[i-2] (its est finished a slot ago),
            # one deferred transpose/proj item, paced fillers, then the
            # scores+exp for seq[i+2]. PE thus never queues an unresolved
            # wait: everything it issues became ready >= 1 slot earlier.
            seq = [(pr, s, kt) for pr in range(NP) for s in range(NS)
                   for kt in range(KVT)]
            STRIP_CAP = globals().get('CAP', 4500)
            ests = {}
            avs = {}

            def emit_scores(pr, s, kt):
                sc = ring_tile()
                qsl = slice(s * 512, (s + 1) * 512)
                for h in range(2):
                    p0, p1 = 64 * h, 64 * h + 64
                    nc.tensor.matmul(
                        sc[:, h, :],
                        lhsT=qk_sb[p0:p1, NP + pr, kt * 128:(kt + 1) * 128],
                        rhs=qk_sb[p0:p1, pr, qsl],
                    )
                if kt in _schrau_kts(pr, s):
                    ei = est_p.tile([128, 2, 512], I16, tag="esti",
                                    bufs=5, name="ei")
                    nc.vector.tensor_scalar(
                        ei, sc, float(SCHRAU_A), float(SCHRAU_B),
                        MULT, ADD)
                    ests[(pr, s, kt)] = ei.bitcast(BF16)
                else:
                    est = est_p.tile([128, 2, 512], BF16, tag="est",
                                     bufs=globals().get("EST_BUFS", 9), name="est")
                    nc.scalar.activation(est, sc, EXP, scale=SCALE)
                    ests[(pr, s, kt)] = est

            def emit_av(pr, s, kt):
                est = ests.pop((pr, s, kt))
                if kt == 0:
                    avs[(pr, s)] = av_p.tile([128, 2, 4, 128], F32,
                                             tag="av", name="av")
                av = avs[(pr, s)]
                for h in range(2):
                    for q2 in range(4):
                        # start only on the bank's first write: a start=True
                        # matmul marks its whole 2KB PSUM bank pending-zero,
                        # so sibling slots must accumulate with start=False
                        nc.tensor.matmul(
                            av[:, h, q2, 0:D + 1],
                            lhsT=est[:, h, q2 * 128:(q2 + 1) * 128],
                            rhs=v_sb[:, kt, 2 * pr + h, :],
                            start=(kt == 0 and q2 == 0),
                            stop=(kt == KVT - 1),
                            skip_group_check=True,
                        )

            def finish_strip(pr, s, last=False):
                """Free the av banks with one copy, then normalize from
                SBUF and queue transposes (+ proj for the last pair).
                For the final strip everything is emitted inline, pipelined
                per q-subtile so the drain chain overlaps."""
                av = avs.pop((pr, s))
                avsb = stg_p.tile([128, 2, 4, D + 1], F32, tag="avsb",
                                  bufs=2, name="avsb")
                nc.vector.tensor_copy(avsb, av[:, :, :, 0:D + 1])
                rec = rec_p.tile([128, 2, 4, 1], F32, tag="rec")
                nc.vector.reciprocal(rec, avsb[:, :, :, D:D + 1])
                stga = stg_p.tile([128, 2, 4, D], BF16, tag="stg",
                                  bufs=2, name="stga")

                def norm_tp(q2):
                    for h in range(2):
                        nc.vector.tensor_scalar(
                            stga[:, h, q2, :], avsb[:, h, q2, 0:D],
                            rec[:, h, q2, :], None, MULT)
                    emit_transpose([stga[:, 0, q2, :], stga[:, 1, q2, :]],
                                   pr, s, q2)

                if last:
                    for q2 in range(4):
                        norm_tp(q2)
                    for mt in (s * 4, s * 4 + 1):
                        for ns_ in range(2):
                            emit_proj_half(mt, ns_, tail=True)
                    for mt in (s * 4 + 2, s * 4 + 3):
                        for ns_ in range(2):
                            emit_proj_half(mt, ns_, tail=True)
                    return
                for q2 in range(4):
                    pending.append((norm_tp, (q2,)))
                if pr == NP - 1:
                    for mt in range(s * 4, s * 4 + 4):
                        for ns_ in range(2):
                            pending.append((emit_proj_half, (mt, ns_)))

            LAG = globals().get('AV_LAG', 4)
            for j in range(LAG):
                emit_scores(*seq[j])
            spent = 0
            for i, trip in enumerate(seq):
                pr, s, kt = trip
                if i >= LAG:
                    emit_av(*seq[i - LAG])
                    opr, os_, okt = seq[i - LAG]
                    if okt == KVT - 1:
                        finish_strip(opr, os_)
                if pending:
                    fn, args = pending.popleft()
                    fn(*args)
                if i + LAG < len(seq):
                    emit_scores(*seq[i + LAG])
                # just-in-time v rows, behind the score stream
                if v0_emitted < KVT and i < KVT:
                    emit_v_mt(0, v0_emitted)
                    v0_emitted += 1
                budget = (i + 1) * STRIP_CAP // KVT
                while fillers and (fillers[0][0] <= i or spent < budget):
                    dl, cyc, fn, args = fillers.popleft()
                    fn(*args)
                    spent += cyc
            for j in range(LAG, 0, -1):
                pj = seq[len(seq) - j]
                if pj[2] == KVT - 1:
                    emit_av(*pj)
                else:
                    emit_av(*pj)
            finish_strip(NP - 1, NS - 1, last=True)
            while pending:
                fn, args = pending.popleft()
                if fn is emit_proj_half:
                    fn(*args, tail=True)
                else:
                    fn(*args)
            while fillers:
                dl, cyc, fn, args = fillers.popleft()
                fn(*args)

    nc.compile()
    return nc


def _get_nc():
    if "nc" not in _CACHE:
        _CACHE["nc"] = _build()
    return _CACHE["nc"]


def _prep_inputs(x, w_qkv, w_proj):
    """Per-core input dicts. Core c: batch c//2, head-half c%2."""
    wq, wk, wv = w_qkv[0:C], w_qkv[C:2 * C], w_qkv[2 * C:3 * C]
    in_maps = []
    for core in range(NCORES):
        b, p = divmod(core, 2)
        heads = [p * HL + j for j in range(HL)]
        qk_rows = np.concatenate(
            [wq[h * D:(h + 1) * D] for h in heads]
            + [wk[h * D:(h + 1) * D] for h in heads], axis=0)   # [768, C]
        v_rows = np.concatenate(
            [wv[h * D:(h + 1) * D] for h in heads], axis=0)     # [384, C]
        p_cols = np.concatenate(
            [w_proj[:, h * D:(h + 1) * D] for h in heads], axis=1)  # [C, 384]
        in_maps.append({
            "xT": np.ascontiguousarray(x[b].T).astype(ml_dtypes.bfloat16),
            "wqkT": np.ascontiguousarray(qk_rows.T).astype(ml_dtypes.bfloat16),
            "wvT": np.ascontiguousarray(v_rows.T).astype(ml_dtypes.bfloat16),
            "wpT": np.ascontiguousarray(p_cols.T).astype(ml_dtypes.bfloat16),
        })
    return in_maps


def kernel(x, w_qkv, w_proj, b_proj, _trace=False):
    x = np.asarray(x, dtype=np.float32)
    w_qkv = np.asarray(w_qkv, dtype=np.float32)
    w_proj = np.asarray(w_proj, dtype=np.float32)
    b_proj = np.asarray(b_proj, dtype=np.float32)

    nc = _get_nc()
    in_maps = _prep_inputs(x, w_qkv, w_proj)
    # The first execution on a cold axon device has been observed to
    # return corrupted results (and the v1 baseline saw transient
    # NRT_EXEC_UNIT_UNRECOVERABLE); always discard a warm-up execution
    # and return the second run's output.
    last_exc = None
    for _attempt in range(3):
        try:
            run_bass_kernel_spmd(nc, in_maps, core_ids=list(range(NCORES)))
            res = run_bass_kernel_spmd(nc, in_maps,
                                       core_ids=list(range(NCORES)),
                                       trace=_trace)
            break
        except Exception as e:
            last_exc = e
    else:
        raise last_exc
    _CACHE["last_results"] = res

    out = np.empty((B, N, C), dtype=np.float32)
    for b in range(B):
        out[b] = (res.results[2 * b]["y"].astype(np.float32)
                  + res.results[2 * b + 1]["y"].astype(np.float32) + b_proj)
    return out


# revision 8
# speedup vs baseline: 1.0351x; 1.0050x over previous
"""Multi-head attention (B=4,N=2048,C=768,H=12) on 8 trn2 NeuronCores.

Sharding: core = (batch b, head-half p): 6 heads of one batch per core;
host sums the two half-partials per batch and adds the bias.

Structure (vs the 281us v1 baseline; now ~236us):
 - AV matmul flipped: out[q,d] = est[kv,q].T @ v[kv,d+1] with est as the
   stationary operand -> out free = 65 instead of 512, so attention@V
   costs 42us of PE instead of 83us. The appended ones column makes the
   softmax denominator fall out of column 64.
 - Normalize is a per-partition tensor_scalar on the [q,d] tile (the
   denominator is a per-q scalar); a PE transpose ([q,d]->[d,q], 128
   cycles, odd head straight into PSUM partitions 64-127 via
   tile_position) rebuilds the proj layout.
 - proj accumulates all 3 head-pairs on-device; y ships as bf16.
 - x/wqkv stream as bf16 (cost model charges the moving operand; f32r
   needs >=256-wide outputs to avoid a 4x penalty) and the input fill is
   7 wide DMAs (the sim serializes a ~625ns HWDGE stage per DMA and all
   transfers share one DMA_ENGINES device).
 - Softmax exp mostly on ACT; a per-strip subset of kv tiles uses a
   Schraudolph bit-trick exp on DVE (bf16(int16(s*A+B)), rms 1.8%,
   zero-mean) to keep ACT off the critical path.
 - Flat software-pipelined slot loop over (pair, strip, kv-tile): slot i
   emits AV for seq[i-3], one deferred normalize/transpose/proj item,
   scores+exp for seq[i+3], then deadline/budget-paced qkproj/v-proj
   fillers, so PE (the pacing engine; idle gaps also downclock it) never
   queues an unresolved wait.

PSUM (8 banks): sc 2x[128,2,512]f32 (4) + av [128,2,4,128]f32 (2,
single buf, 512B slot stride so no accumulation group crosses a 2KB
bank; start=True only on each bank's first write since it marks the
whole bank pending-zero) + tp [128,128]bf16 (1) + scratch [128,512]f32
(1, shared by warmup/qkproj/v/proj via one tag).
"""

import sys
from collections import deque

import numpy as np
import ml_dtypes

_REPO = "/opt/trn_rl_repo"
if _REPO not in sys.path:
    sys.path.insert(0, _REPO)

import concourse.bacc as bacc
import concourse.mybir as mybir
import concourse.tile as tile
from concourse.bass_utils import run_bass_kernel_spmd
from concourse.masks import make_identity

B, N, C, H, D = 4, 2048, 768, 12, 64
HL = H // 2          # heads per core
NP = HL // 2         # head pairs per core (3)
SCALE = D ** -0.5
NCORES = 8
KT_C = C // 128      # 6 contraction tiles over C
KVT = N // 128       # 16 kv tiles
NS = N // 512        # 4 query strips of 512

F32 = mybir.dt.float32
F32R = mybir.dt.float32r
BF16 = mybir.dt.bfloat16
I16 = mybir.dt.int16
EXP = mybir.ActivationFunctionType.Exp
MULT = mybir.AluOpType.mult
ADD = mybir.AluOpType.add

# Schraudolph bit-trick exp on DVE: bf16(int16(s*A + B)) ~ exp(s*SCALE).
# rms rel err 1.8%, near-zero mean (c=-7.5); used on a few kv tiles per
# strip to keep ACT off the critical path (PE must stay saturated: any
# PE idle gap drops its clock to 1.2GHz for the next 3us).
SCHRAU_A = 128 * np.log2(np.e) * SCALE
SCHRAU_B = 127.0 * 128 - 7.5
# kv tiles whose exp runs on DVE instead of ACT, by (pair, strip):
# pair0 is PE-overloaded (v-proj + deadline qk chunks), so no offload
# there; pair1/pair2 strips offload 2-3 tiles to keep ACT off the
# critical path while PE stays saturated
SCHRAU_PLAN = {0: ((4, 11),) * 4,
               1: ((0, 2, 5, 8, 11, 14),) * 4,
               2: ((0, 2, 5, 8, 11, 14),) * 4}


def _schrau_kts(pr, s):
    return SCHRAU_PLAN[pr][s]

_CACHE = {}


def _build():
    nc = bacc.Bacc("TRN2", target_bir_lowering=False, debug=False,
                   num_devices=NCORES)
    xT = nc.dram_tensor("xT", [C, N], BF16, kind="ExternalInput").ap()
    wqkT = nc.dram_tensor("wqkT", [C, 2 * HL * D], BF16, kind="ExternalInput").ap()
    wvT = nc.dram_tensor("wvT", [C, HL * D], BF16, kind="ExternalInput").ap()
    wpT = nc.dram_tensor("wpT", [HL * D, C], BF16, kind="ExternalInput").ap()
    y = nc.dram_tensor("y", [N, C], BF16, kind="ExternalOutput").ap()

    with tile.TileContext(nc) as tc:
        with (
            tc.tile_pool(name="singles", bufs=1) as singles,
            tc.tile_pool(name="sc_p", bufs=3, space="PSUM") as sc_p,
            tc.tile_pool(name="av_p", bufs=1, space="PSUM") as av_p,
            tc.tile_pool(name="est_p", bufs=4) as est_p,
            tc.tile_pool(name="stg_p", bufs=6) as stg_p,
            tc.tile_pool(name="rec_p", bufs=2) as rec_p,
            tc.tile_pool(name="ysb_p", bufs=3) as ysb_p,
        ):
            xT_sb = singles.tile([128, KT_C, N], BF16)
            wqk_sb = singles.tile([128, KT_C, 2 * HL * D], BF16)
            wv_sb = singles.tile([128, KT_C, HL * D], BF16)
            wp_sb = singles.tile([128, NP, C], BF16)
            qk_sb = singles.tile([128, 2 * NP, N], F32R)
            v_sb = singles.tile([128, KVT, HL, D + 1], BF16)
            attn_sb = singles.tile([128, NP, N], BF16)
            ident = singles.tile([128, 128], BF16)
            warm_sb = singles.tile([128, 640], BF16)

            # The sim serializes a ~625ns HWDGE stage per DMA instruction,
            # so batch the fill into 7 wide DMAs. xT goes chunk-major on
            # SP's queue (first score work unblocks after ~3.5us); weights
            # go on ACT's queue.
            xT_r = xT.rearrange("(k p) n -> p k n", k=KT_C)
            wqk_r = wqkT.rearrange("(k p) n -> p k n", k=KT_C)
            nc.sync.dma_start(wqk_sb[:, :, HL * D:], wqk_r[:, :, HL * D:])
            nc.sync.dma_start(xT_sb[:, :, 0:512], xT_r[:, :, 0:512])
            nc.sync.dma_start(wqk_sb[:, :, 0:HL * D], wqk_r[:, :, 0:HL * D])
            nc.sync.dma_start(wv_sb, wvT.rearrange("(k p) n -> p k n", k=KT_C))
            for c in range(1, NS):
                csl = slice(c * 512, (c + 1) * 512)
                nc.sync.dma_start(xT_sb[:, :, csl], xT_r[:, :, csl])
            nc.sync.dma_start(wp_sb, wpT.rearrange("(k p) n -> p k n", k=NP))
            nc.vector.memset(warm_sb, 0.0)
            nc.vector.memset(v_sb[:, :, :, D:D + 1], 1.0)
            make_identity(nc, ident)

            # warm the ACT exp table (hardware-only cost; sim ignores it)
            warm_in = rec_p.tile([1, 2], F32, tag="warm")
            warm_out = rec_p.tile([1, 2], BF16, tag="warmo")
            nc.vector.memset(warm_in, 0.0)
            nc.scalar.activation(warm_out, warm_in, EXP, scale=SCALE)

            def ring_tile():
                t = sc_p.tile([128, 2, 512], F32, tag="sc", name="sc")
                return t

            # PE p-state ramp warmers while the xT DMA fill lands
            for _ in range(10):
                warm_ps = ring_tile()
                nc.tensor.matmul(warm_ps[:, 0, :], lhsT=warm_sb[:, 0:128],
                                 rhs=warm_sb[:, 128:640])

            # ---- filler emission (qkproj / v-proj paced into attention) ----
            def emit_qk_chunk(t, c, via_act=False):
                """qk tile t (0-2: q pairs, 3-5: k pairs), 512-col chunk c.
                via_act stages the PSUM->SBUF copy on the scalar engine,
                which has slack in the exp-offloaded pairs, keeping DVE's
                queue short (the offloaded exp sits in the score-ring WAR
                chain, so DVE latency there stalls PE)."""
                ps = ring_tile()[:, 0, :]
                csl = slice(c * 512, (c + 1) * 512)
                for kt in range(KT_C):
                    nc.tensor.matmul(
                        ps,
                        lhsT=wqk_sb[:, kt, t * 128:(t + 1) * 128],
                        rhs=xT_sb[:, kt, csl],
                        start=(kt == 0), stop=(kt == KT_C - 1),
                    )
                if via_act:
                    nc.scalar.copy(qk_sb[:, t, csl], ps)
                else:
                    nc.vector.tensor_copy(qk_sb[:, t, csl], ps)

            def emit_v_mt(pr, mt):
                """v rows for kv tile mt, all 6 heads (pr unused; f32r
                needs a >=256-wide moving operand to stream 1 cycle/row)."""
                ps = ring_tile()[:, 0, :]
                for kt in range(KT_C):
                    nc.tensor.matmul(
                        ps[:, 0:384],
                        lhsT=xT_sb[:, kt, mt * 128:(mt + 1) * 128],
                        rhs=wv_sb[:, kt, :],
                        start=(kt == 0), stop=(kt == KT_C - 1),
                    )
                nc.vector.tensor_copy(
                    v_sb[:, mt, :, 0:D],
                    ps[:, 0:384].rearrange("p (h d) -> p h d", h=HL),
                )

            # fillers carry a deadline slot: q-tile chunk c of pair pr is
            # read starting at slot 16*(4*pr+c)-2; k-tile chunk c at slot
            # 64*pr+4*c-2. Budget pacing pulls them earlier when PE has
            # slack; the deadline forces emission when it hasn't.
            QK_CYC, V_CYC = KT_C * 512 + 533, KT_C * 384 + 533
            items = []
            for c in (1, 2, 3):
                # k chunks of pair0, consumed by sc(0,0,4c) at slot 4c-2;
                # paced to match the chunk-major xT DMA arrivals
                items.append((4 * c - 3, QK_CYC, emit_qk_chunk, (3, c)))
                items.append((16 * c - 4, QK_CYC, emit_qk_chunk, (0, c)))
            for c in range(NS):
                items.append((58 + 16 * c, QK_CYC, emit_qk_chunk, (1, c)))
                items.append((52 + 4 * c, QK_CYC, emit_qk_chunk, (4, c)))
                items.append((100 + 4 * c, QK_CYC, emit_qk_chunk, (2, c)))
                items.append((96 + 2 * c, QK_CYC, emit_qk_chunk, (5, c)))
            items.sort(key=lambda it: it[0])
            fillers = deque(items)

            # deferred work carried across strips: transposes of the
            # previous strip (list of fns), proj of the previous strip
            pending = deque()

            def emit_transpose(stg01, pr, s, q2):
                """Transpose both heads' [q,64] tiles into one [128,128]
                psum tile (h1 straight to partitions 64-127), then copy."""
                tp = ring_tile()[:, 0, 0:64].bitcast(BF16)
                for h in range(2):
                    out = tp[64 * h:64 * h + 64, :]
                    nc.tensor.matmul(out, lhsT=stg01[h], rhs=ident,
                                     is_transpose=True,
                                     tile_position=(0, 64 * h))
                csl = slice(s * 512 + q2 * 128, s * 512 + (q2 + 1) * 128)
                nc.vector.tensor_copy(attn_sb[:, pr, csl], tp)

            proj_ysb = {}
            ysb_tail = singles.tile([128, 4, 2, 384], BF16, name="ysb_tail")

            def emit_proj_half(mt, ns_, tail=False):
                if ns_ == 0 and not tail:
                    ysb_new = ysb_p.tile([128, 2, 384], BF16, tag="ysb",
                                         name="ysb")
                    proj_ysb[mt] = ysb_new
                ysb = proj_ysb.get(mt)
                yp = ring_tile()[:, 0, :]
                for pr in range(NP):
                    nc.tensor.matmul(
                        yp[:, 0:384],
                        lhsT=attn_sb[:, pr, mt * 128:(mt + 1) * 128],
                        rhs=wp_sb[:, pr, ns_ * 384:(ns_ + 1) * 384],
                        start=(pr == 0), stop=(pr == NP - 1),
                    )
                if tail:
                    mi = mt - (NS - 1) * 4
                    nc.scalar.copy(ysb_tail[:, mi, ns_, :], yp[:, 0:384])
                    if mi == 1 and ns_ == 1:
                        nc.sync.dma_start(
                            y.rearrange("(m p) c -> p m c", m=KVT)[:, 12:14, :],
                            ysb_tail[:, 0:2].rearrange("p m a b -> p m (a b)"))
                    elif mi == 3 and ns_ == 1:
                        nc.sync.dma_start(
                            y.rearrange("(m p) c -> p m c", m=KVT)[:, 14:16, :],
                            ysb_tail[:, 2:4].rearrange("p m a b -> p m (a b)"))
                    return
                nc.vector.tensor_copy(ysb[:, ns_, :], yp[:, 0:384])
                if ns_ == 1:
                    nc.sync.dma_start(y[mt * 128:(mt + 1) * 128, :],
                                      ysb.rearrange("p a b -> p (a b)"))
                    del proj_ysb[mt]

            # ---- prefill: only chunk-0-dependent work, so the first
            # scores can issue as soon as the first 6 xT DMAs land ----
            emit_qk_chunk(3, 0)              # k tile of pair0, kv 0-3
            emit_qk_chunk(0, 0)              # q tile of pair0, strip0
            v0_emitted = 0

            # ---- attention: flat software-pipelined slot loop ----
            # slot i emits: av for seq[i-2] (its est finished a slot ago),
            # one deferred transpose/proj item, paced fillers, then the
            # scores+exp for seq[i+2]. PE thus never queues an unresolved
            # wait: everything it issues became ready >= 1 slot earlier.
            seq = [(pr, s, kt) for pr in range(NP) for s in range(NS)
                   for kt in range(KVT)]
            STRIP_CAP = globals().get('CAP', 4500)
            ests = {}
            avs = {}

            def emit_scores(pr, s, kt):
                sc = ring_tile()
                qsl = slice(s * 512, (s + 1) * 512)
                for h in range(2):
                    p0, p1 = 64 * h, 64 * h + 64
                    nc.tensor.matmul(
                        sc[:, h, :],
                        lhsT=qk_sb[p0:p1, NP + pr, kt * 128:(kt + 1) * 128],
                        rhs=qk_sb[p0:p1, pr, qsl],
                    )
                if kt in _schrau_kts(pr, s):
                    ei = est_p.tile([128, 2, 512], I16, tag="esti",
                                    bufs=5, name="ei")
                    nc.vector.tensor_scalar(
                        ei, sc, float(SCHRAU_A), float(SCHRAU_B),
                        MULT, ADD)
                    ests[(pr, s, kt)] = ei.bitcast(BF16)
                else:
                    est = est_p.tile([128, 2, 512], BF16, tag="est",
                                     bufs=globals().get("EST_BUFS", 9), name="est")
                    nc.scalar.activation(est, sc, EXP, scale=SCALE)
                    ests[(pr, s, kt)] = est

            def emit_av(pr, s, kt):
                est = ests.pop((pr, s, kt))
                if kt == 0:
                    avs[(pr, s)] = av_p.tile([128, 2, 4, 128], F32,
                                             tag="av", name="av")
                av = avs[(pr, s)]
                for h in range(2):
                    for q2 in range(4):
                        # start only on the bank's first write: a start=True
                        # matmul marks its whole 2KB PSUM bank pending-zero,
                        # so sibling slots must accumulate with start=False
                        nc.tensor.matmul(
                            av[:, h, q2, 0:D + 1],
                            lhsT=est[:, h, q2 * 128:(q2 + 1) * 128],
                            rhs=v_sb[:, kt, 2 * pr + h, :],
                            start=(kt == 0 and q2 == 0),
                            stop=(kt == KVT - 1),
                            skip_group_check=True,
                        )

            def finish_strip(pr, s, last=False):
                """Free the av banks with one copy, then normalize from
                SBUF and queue transposes (+ proj for the last pair).
                For the final strip everything is emitted inline, pipelined
                per q-subtile so the drain chain overlaps."""
                av = avs.pop((pr, s))
                avsb = stg_p.tile([128, 2, 4, D + 1], F32, tag="avsb",
                                  bufs=2, name="avsb")
                nc.vector.tensor_copy(avsb, av[:, :, :, 0:D + 1])
                rec = rec_p.tile([128, 2, 4, 1], F32, tag="rec")
                nc.vector.reciprocal(rec, avsb[:, :, :, D:D + 1])
                stga = stg_p.tile([128, 2, 4, D], BF16, tag="stg",
                                  bufs=2, name="stga")

                def norm_tp(q2):
                    for h in range(2):
                        nc.vector.tensor_scalar(
                            stga[:, h, q2, :], avsb[:, h, q2, 0:D],
                            rec[:, h, q2, :], None, MULT)
                    emit_transpose([stga[:, 0, q2, :], stga[:, 1, q2, :]],
                                   pr, s, q2)

                if last:
                    for q2 in range(4):
                        norm_tp(q2)
                    for mt in (s * 4, s * 4 + 1):
                        for ns_ in range(2):
                            emit_proj_half(mt, ns_, tail=True)
                    for mt in (s * 4 + 2, s * 4 + 3):
                        for ns_ in range(2):
                            emit_proj_half(mt, ns_, tail=True)
                    return
                for q2 in range(4):
                    pending.append((norm_tp, (q2,)))
                if pr == NP - 1:
                    for mt in range(s * 4, s * 4 + 4):
                        for ns_ in range(2):
                            pending.append((emit_proj_half, (mt, ns_)))

            LAG = globals().get('AV_LAG', 4)
            for j in range(LAG):
                emit_scores(*seq[j])
            spent = 0
            for i, trip in enumerate(seq):
                pr, s, kt = trip
                if i >= LAG:
                    emit_av(*seq[i - LAG])
                    opr, os_, okt = seq[i - LAG]
                    if okt == KVT - 1:
                        finish_strip(opr, os_)
                if pending:
                    fn, args = pending.popleft()
                    fn(*args)
                if i + LAG < len(seq):
                    emit_scores(*seq[i + LAG])
                # just-in-time v rows, behind the score stream
                if v0_emitted < KVT and i < KVT:
                    emit_v_mt(0, v0_emitted)
                    v0_emitted += 1
                budget = (i + 1) * STRIP_CAP // KVT
                while fillers and (fillers[0][0] <= i or spent < budget):
                    dl, cyc, fn, args = fillers.popleft()
                    fn(*args)
                    spent += cyc
            for j in range(LAG, 0, -1):
                pj = seq[len(seq) - j]
                if pj[2] == KVT - 1:
                    emit_av(*pj)
                else:
                    emit_av(*pj)
            finish_strip(NP - 1, NS - 1, last=True)
            while pending:
                fn, args = pending.popleft()
                if fn is emit_proj_half:
                    fn(*args, tail=True)
                else:
                    fn(*args)
            while fillers:
                dl, cyc, fn, args = fillers.popleft()
                fn(*args)

    nc.compile()
    return nc


def _get_nc():
    if "nc" not in _CACHE:
        _CACHE["nc"] = _build()
    return _CACHE["nc"]


def _prep_inputs(x, w_qkv, w_proj):
    """Per-core input dicts. Core c: batch c//2, head-half c%2."""
    wq, wk, wv = w_qkv[0:C], w_qkv[C:2 * C], w_qkv[2 * C:3 * C]
    in_maps = []
    for core in range(NCORES):
        b, p = divmod(core, 2)
        heads = [p * HL + j for j in range(HL)]
        qk_rows = np.concatenate(
            [wq[h * D:(h + 1) * D] for h in heads]
            + [wk[h * D:(h + 1) * D] for h in heads], axis=0)   # [768, C]
        v_rows = np.concatenate(
            [wv[h * D:(h + 1) * D] for h in heads], axis=0)     # [384, C]
        p_cols = np.concatenate(
            [w_proj[:, h * D:(h + 1) * D] for h in heads], axis=1)  # [C, 384]
        in_maps.append({
            "xT": np.ascontiguousarray(x[b].T).astype(ml_dtypes.bfloat16),
            "wqkT": np.ascontiguousarray(qk_rows.T).astype(ml_dtypes.bfloat16),
            "wvT": np.ascontiguousarray(v_rows.T).astype(ml_dtypes.bfloat16),
            "wpT": np.ascontiguousarray(p_cols.T).astype(ml_dtypes.bfloat16),
        })
    return in_maps


def kernel(x, w_qkv, w_proj, b_proj, _trace=False):
    x = np.asarray(x, dtype=np.float32)
    w_qkv = np.asarray(w_qkv, dtype=np.float32)
    w_proj = np.asarray(w_proj, dtype=np.float32)
    b_proj = np.asarray(b_proj, dtype=np.float32)

    nc = _get_nc()
    in_maps = _prep_inputs(x, w_qkv, w_proj)
    # The first execution on a cold axon device has been observed to
    # return corrupted results (and the v1 baseline saw transient
    # NRT_EXEC_UNIT_UNRECOVERABLE); always discard a warm-up execution
    # and return the second run's output.
    last_exc = None
    for _attempt in range(3):
        try:
            run_bass_kernel_spmd(nc, in_maps, core_ids=list(range(NCORES)))
            res = run_bass_kernel_spmd(nc, in_maps,
                                       core_ids=list(range(NCORES)),
                                       trace=_trace)
            break
        except Exception as e:
            last_exc = e
    else:
        raise last_exc
    _CACHE["last_results"] = res

    out = np.empty((B, N, C), dtype=np.float32)
    for b in range(B):
        out[b] = (res.results[2 * b]["y"].astype(np.float32)
                  + res.results[2 * b + 1]["y"].astype(np.float32) + b_proj)
    return out
